# revision 14
# baseline (speedup 1.0000x reference)
"""Trainium2 Bass kernel for nn_CFDFVnewGCN (6-layer FVnewConv GNN).

Strategy: shard destination nodes (and their incoming edges) across 8 cores.
Nodes are permuted/degree-balanced into 49 windows of <=128 nodes per core.
fp16 datapath end-to-end (PSUM accumulation stays f32).  Per 128-edge tile:
scaling matmul (edge_attr stationary, bias folded as 8th K-row) into PSUM,
fused relu*x_j via scalar_tensor_tensor (PSUM -> fp16 msg, split between DVE
and GPSIMD), one-hot scatter matmul accumulating aggr[window, D] in PSUM.
Per window: PE transposes of aggr + output matmul (bias via const ones row),
tanh (ACT) + relu (DVE).  d-layout: 3 planes of 512 gathered x cols
(d = h*512 + i), then a small tail (na, and fyo for c0) so the scatter PSUM
is exactly 4 banks.  Software pipeline: at iteration `it` we emit
gather(it) / scaling+msg(it-1) interleaved with scatter(it-2) /
transposes(it-3) / out-matmul(it-4), keeping the PE stream dependency-free
and continuously busy (full 2.4 GHz p-state).  x replicated via 4 chunked
AllGathers per layer (overlapping compute) into a Shared DRAM buffer.
Gather uses dma_gather with two offset views of the x buffer (rows 0:32768
and 17232:50000) so indices fit int16.
"""
import sys
import numpy as np

for _p in ('/opt/trn_rl_repo', '/root/.axon_site/_ro/trn_rl_repo'):
    if _p not in sys.path:
        sys.path.insert(0, _p)

import concourse.bacc as bacc
import concourse.mybir as mybir
import concourse.tile as tile
from concourse.bass_utils import run_bass_kernel_spmd

F32 = mybir.dt.float32
F16 = mybir.dt.float16
I16 = mybir.dt.int16
I32 = mybir.dt.int32
COPY = mybir.ActivationFunctionType.Copy
RELU = mybir.ActivationFunctionType.Relu
TANH = mybir.ActivationFunctionType.Tanh
MULT = mybir.AluOpType.mult
MAX = mybir.AluOpType.max
ISEQ = mybir.AluOpType.is_equal

NCORES = 8
G = 512          # gathered x columns (one plane width)
DMAIN = 3 * G    # 1536


class Cfg:
    def __init__(self, n_nodes=50000, n_edges=200000, hid=512, hs=3, ea=6,
                 out=3, n_ag_chunks=4):
        self.N = n_nodes
        self.E = n_edges
        self.HID = hid
        self.HS = hs
        self.EA = ea
        self.OUT = out
        self.NPC = self.N // NCORES              # nodes per core
        self.NWIN = (self.NPC + 127) // 128      # windows per core
        self.WSIZES = [128] * (self.NWIN - 1) + [self.NPC - 128 * (self.NWIN - 1)]
        # A/B view split of the x buffer rows (int16 gather index range)
        self.VIEW = min(32768, self.N)
        self.ABOFS = max(0, self.N - self.VIEW)
        # AllGather chunking: split windows into n_ag_chunks groups
        k = min(n_ag_chunks, self.NWIN)
        bounds = [round(i * self.NWIN / k) for i in range(k + 1)]
        self.WCHUNKS = [(bounds[i], bounds[i + 1]) for i in range(k)]
        self.CROWS = [sum(self.WSIZES[a:b]) for a, b in self.WCHUNKS]
        # layer table
        #  p0: ic=7  ([x5,sdf,na] + pad), gathered from xc0 (16 fp16 cols)
        #  others: 3 planes of 512 x-cols + tail (na=3; c0 adds fyo=9)
        self.LAYERS = []
        for name in ['p0', 'p1', 'p2', 'c0', 'c1', 'c2']:
            if name == 'p0':
                lay = dict(name=name, oc=hid, dmain=0, tw=24, gcols=128,
                           relu=True)
            else:
                oc = out if name == 'c2' else hid
                tw = 12 if name == 'c0' else 4
                lay = dict(name=name, oc=oc, dmain=DMAIN, tw=tw, gcols=G,
                           relu=(name != 'c2'))
            lay['D'] = lay['dmain'] + lay['tw']
            lay['OCP'] = lay['oc'] + (-lay['oc']) % 4
            self.LAYERS.append(lay)


def _col2orig(cfg, lay):
    """Map new d-layout column -> original scaling index j=i*HS+h, -1=pad."""
    HS = cfg.HS
    m = np.full(lay['D'], -1, np.int64)
    nm = lay['name']
    if nm == 'p0':
        # d = h*8 + i, i<7 ([x5, sdf, na])
        for h in range(HS):
            for i in range(7):
                m[h * 8 + i] = i * HS + h
    else:
        # ic layout in reference xc: c0: [fyo3, x512, na1]; else [x512, na1]
        xoff = 3 if nm == 'c0' else 0
        for h in range(HS):
            for i in range(G):
                m[h * G + i] = (xoff + i) * HS + h
        t0 = DMAIN
        if nm == 'c0':
            for f in range(3):
                for h in range(HS):
                    m[t0 + 3 * f + h] = f * HS + h        # fyo
            for h in range(HS):
                m[t0 + 9 + h] = (3 + G) * HS + h          # na
        else:
            for h in range(HS):
                m[t0 + h] = G * HS + h                    # na
    return m


def _balance(items_deg, caps):
    """Greedy: assign items (sorted by degree desc) to bins with capacity,
    minimizing max degree sum. Returns bin index per item."""
    order = np.argsort(-items_deg, kind='stable')
    nbins = len(caps)
    load = np.zeros(nbins)
    cnt = np.zeros(nbins, np.int64)
    out = np.zeros(len(items_deg), np.int64)
    import heapq
    heap = [(0.0, b) for b in range(nbins)]
    heapq.heapify(heap)
    for it in order:
        while True:
            l, b = heapq.heappop(heap)
            if cnt[b] < caps[b]:
                break
        out[it] = b
        cnt[b] += 1
        load[b] += items_deg[it]
        if cnt[b] < caps[b]:
            heapq.heappush(heap, (load[b], b))
    return out


def _preprocess(cfg, inputs):
    N, E, HS = cfg.N, cfg.E, cfg.HS
    ei = np.asarray(inputs['edge_index'])
    src = ei[0].astype(np.int64)
    dst = ei[1].astype(np.int64)
    deg = np.bincount(dst, minlength=N).astype(np.float64)

    node_core = _balance(deg, [cfg.NPC] * NCORES)
    node_win = np.zeros(N, np.int64)
    node_slot = np.zeros(N, np.int64)
    for c in range(NCORES):
        nodes = np.where(node_core == c)[0]
        w = _balance(deg[nodes], cfg.WSIZES)
        node_win[nodes] = w
        for wi in range(cfg.NWIN):
            sel = nodes[w == wi]
            node_slot[sel] = np.arange(len(sel))

    # within-core row and global x row (AG chunk-major, rank-interleaved)
    node_row = node_win * 128 + node_slot
    cbase = np.concatenate([[0], np.cumsum([r * NCORES for r in cfg.CROWS])])
    wchunk = np.zeros(cfg.NWIN, np.int64)
    wofs = np.zeros(cfg.NWIN, np.int64)
    for k, (a, b) in enumerate(cfg.WCHUNKS):
        for w in range(a, b):
            wchunk[w] = k
            wofs[w] = sum(cfg.WSIZES[a:w])
    k_of = wchunk[node_win]
    xrow = (cbase[k_of] + node_core * np.array(cfg.CROWS)[k_of]
            + wofs[node_win] + node_slot)
    xrow_src = xrow[src]

    # edge buckets per (core, window)
    ec = node_core[dst]
    ew = node_win[dst]
    # forced side by xrow of src
    fA = xrow_src < cfg.ABOFS
    fB = xrow_src >= cfg.VIEW

    # per-window global tile structure (max over cores)
    kA = np.zeros(cfg.NWIN, np.int64)
    kB = np.zeros(cfg.NWIN, np.int64)
    tw = np.zeros(cfg.NWIN, np.int64)
    cntA = np.zeros((NCORES, cfg.NWIN), np.int64)
    cntB = np.zeros((NCORES, cfg.NWIN), np.int64)
    cntT = np.zeros((NCORES, cfg.NWIN), np.int64)
    np.add.at(cntA, (ec[fA], ew[fA]), 1)
    np.add.at(cntB, (ec[fB], ew[fB]), 1)
    np.add.at(cntT, (ec, ew), 1)
    for w in range(cfg.NWIN):
        ka = int(np.ceil(cntA[:, w].max() / 128))
        kb = int(np.ceil(cntB[:, w].max() / 128))
        t = max(ka + kb, int(np.ceil(cntT[:, w].max() / 128)), 1)
        kA[w] = ka
        kB[w] = t - ka
        tw[w] = t
        assert kB[w] >= kb
    tbase = np.concatenate([[0], np.cumsum(tw)])
    T = int(tbase[-1])

    ea_np = np.asarray(inputs['edge_attr'], np.float32)
    na_np = np.asarray(inputs['node_attr'], np.float32).reshape(-1)
    fyo_np = np.asarray(inputs['fine_y_orig'], np.float32)

    per_core = []
    for c in range(NCORES):
        ea_s = np.zeros((7, T * 128), np.float16)
        idx_s = np.zeros((16, T * 8), np.int16)
        dst_s = np.full((128, T), 999.0, np.float16)
        na_s = np.zeros((128, T), np.float16)
        fyo_s = np.zeros((128, 3 * T), np.float16)
        for w in range(cfg.NWIN):
            eidx = np.where((ec == c) & (ew == w))[0]
            if len(eidx):
                a_e = eidx[fA[eidx]]
                m_e = eidx[~fA[eidx] & ~fB[eidx]]
                b_e = eidx[fB[eidx]]
                capA = int(kA[w]) * 128
                take = min(len(m_e), capA - len(a_e))
                A = np.concatenate([a_e, m_e[:take]])
                B = np.concatenate([b_e, m_e[take:]])
            else:
                A = B = np.array([], np.int64)
            assert len(A) <= kA[w] * 128 and len(B) <= kB[w] * 128, (w, len(A), len(B))
            for side, edges, ktiles, t0 in (
                    (0, A, int(kA[w]), int(tbase[w])),
                    (1, B, int(kB[w]), int(tbase[w] + kA[w]))):
                nslots = ktiles * 128
                if nslots == 0:
                    continue
                iv = np.zeros(nslots, np.int64)
                iv[:len(edges)] = xrow_src[edges] - (0 if side == 0 else cfg.ABOFS)
                assert iv.min() >= 0 and iv.max() < 32768, (iv.min(), iv.max())
                jj = np.arange(nslots)
                tt = t0 + jj // 128
                pp = jj % 128
                idx_s[jj % 16, t0 * 8 + jj // 16] = iv.astype(np.int16)
                if len(edges):
                    e_jj = jj[:len(edges)]
                    e_tt = tt[:len(edges)]
                    e_pp = pp[:len(edges)]
                    ea_s[0:6, e_tt * 128 + e_pp] = ea_np[edges].T
                    ea_s[6, e_tt * 128 + e_pp] = 1.0
                    dst_s[e_pp, e_tt] = node_slot[dst[edges]]
                    na_s[e_pp, e_tt] = na_np[src[edges]]
                    fyo_s[e_pp.repeat(3), (e_tt * 3).repeat(3)
                          + np.tile([0, 1, 2], len(edges))] = fyo_np[src[edges]].ravel()
        per_core.append(dict(ea_s=ea_s, idx_s=np.tile(idx_s, (8, 1)),
                             dst_s=dst_s, na_s=na_s, fyo_s=fyo_s))

    # xc0 buffer: [N, 16] fp16 in x-row order: cols [x(5), sdf, na, 0...]
    x_np = np.asarray(inputs['x'], np.float32)
    sdf_np = np.asarray(inputs['sdf'], np.float32)
    xc0 = np.zeros((N, 128), np.float16)
    xc0[xrow, 0:x_np.shape[1]] = x_np
    xc0[xrow, x_np.shape[1]] = sdf_np[:, 0]
    xc0[xrow, x_np.shape[1] + 1] = na_np

    # weights per layer (fp16, reordered to kernel d-layout)
    wts = {}
    for lay in cfg.LAYERS:
        nm = lay['name']
        win = np.asarray(inputs[f'win_{nm}'], np.float32)
        bin_ = np.asarray(inputs[f'bin_{nm}'], np.float32)
        wout = np.asarray(inputs[f'wout_{nm}'], np.float32)
        bout = np.asarray(inputs[f'bout_{nm}'], np.float32)
        m = _col2orig(cfg, lay)
        D, OCP = lay['D'], lay['OCP']
        winT = np.zeros((7, D), np.float16)
        sel = m >= 0
        winT[0:cfg.EA, sel] = win[m[sel]].T
        winT[6, sel] = bin_[m[sel]]
        woutT = np.zeros((D + 1, OCP), np.float16)
        woutT[np.where(sel)[0], 0:lay['oc']] = wout[:, m[sel]].T
        woutT[D, 0:lay['oc']] = bout
        wts[f'winT_{nm}'] = winT
        wts[f'woutT_{nm}'] = woutT

    struct = dict(kA=kA, kB=kB, tw=tw, tbase=tbase, T=T,
                  TWMAX=int(tw.max()))
    asm = dict(node_core=node_core, node_row=node_row)
    return struct, per_core, wts, xc0, asm


def _build(cfg, struct):
    kA, kB, twin, tbase, T = (struct['kA'], struct['kB'], struct['tw'],
                              struct['tbase'], struct['T'])
    TWMAX = struct['TWMAX']
    HID = cfg.HID
    NW = cfg.NWIN
    DMAX = max(l['D'] for l in cfg.LAYERS)

    nc = bacc.Bacc("TRN2", target_bir_lowering=False, debug=False,
                   enable_asserts=True, num_devices=NCORES,
                   num_swdge_queues=4)
    ea_in = nc.dram_tensor("ea_s", [7, T * 128], F16, kind="ExternalInput").ap()
    idx_in = nc.dram_tensor("idx_s", [128, T * 8], I16, kind="ExternalInput").ap()
    dst_in = nc.dram_tensor("dst_s", [128, T], F16, kind="ExternalInput").ap()
    na_in = nc.dram_tensor("na_s", [128, T], F16, kind="ExternalInput").ap()
    fyo_in = nc.dram_tensor("fyo_s", [128, 3 * T], F16, kind="ExternalInput").ap()
    xc0_in = nc.dram_tensor("xc0_in", [cfg.N, 128], F16, kind="ExternalInput").ap()
    win_ins = {}
    wout_ins = {}
    for lay in cfg.LAYERS:
        nm = lay['name']
        win_ins[nm] = nc.dram_tensor(f"winT_{nm}", [7, lay['D']], F16,
                                     kind="ExternalInput").ap()
        wout_ins[nm] = nc.dram_tensor(f"woutT_{nm}", [lay['D'] + 1, lay['OCP']],
                                      F16, kind="ExternalInput").ap()
    out_fin = nc.dram_tensor("out_final", [cfg.NPC, cfg.OUT], F32,
                             kind="ExternalOutput").ap()

    with tile.TileContext(nc) as tc:
        with (
            tc.tile_pool(name="cst", bufs=1) as cst,
            tc.tile_pool(name="sbw", bufs=2) as sbw,
            tc.tile_pool(name="gst", bufs=3) as gst,
            tc.tile_pool(name="eap", bufs=3) as eap,
            tc.tile_pool(name="msgp", bufs=2) as msgp,
            tc.tile_pool(name="scp", bufs=3) as scp,
            tc.tile_pool(name="Sp", bufs=3) as Sp,
            tc.tile_pool(name="agsp", bufs=2) as agsp,
            tc.tile_pool(name="agtp", bufs=2) as agtp,
            tc.tile_pool(name="outp", bufs=2) as outp,
            tc.tile_pool(name="ps_sc", bufs=2, space="PSUM") as ps_sc,
            tc.tile_pool(name="ps_ag", bufs=1, space="PSUM") as ps_ag,
            tc.tile_pool(name="ps_tp", bufs=2, space="PSUM") as ps_tp,
            tc.tile_pool(name="dram", bufs=1, space="DRAM") as dram,
        ):
            # ---- constants
            iota_i = cst.tile([128, 128], I32)
            nc.gpsimd.iota(iota_i[:, :], pattern=[[1, 128]], base=0,
                           channel_multiplier=0)
            iota_f = cst.tile([128, 128], F16)
            nc.vector.tensor_copy(iota_f[:, :], iota_i[:, :])
            iota_p = cst.tile([128, 1], I32)
            nc.gpsimd.iota(iota_p[:, :], pattern=[[1, 1]], base=0,
                           channel_multiplier=1)
            iota_pf = cst.tile([128, 1], F32)
            nc.vector.tensor_copy(iota_pf[:, :], iota_p[:, :])
            identf = cst.tile([128, 128], F32)
            nc.vector.tensor_scalar(out=identf[:, :], in0=iota_f[:, :],
                                    scalar1=iota_pf[:, :], scalar2=None,
                                    op0=ISEQ)
            ident = cst.tile([128, 128], F16)
            nc.vector.tensor_copy(ident[:, :], identf[:, :])
            ones_i = cst.tile([1, 128], I32)
            nc.gpsimd.iota(ones_i[:, :], pattern=[[0, 128]], base=1,
                           channel_multiplier=0)
            ones = cst.tile([1, 128], F16)
            nc.vector.tensor_copy(ones[:, :], ones_i[:, :])
            zeros = cst.tile([128, 512], F16)
            nc.vector.memset(zeros[:, :], 0)

            # ---- static per-slot data (resident)
            dst_sb = cst.tile([128, T], F16)
            nc.sync.dma_start(out=dst_sb[:, :], in_=dst_in[:, :])
            na_sb = cst.tile([128, T], F16)
            nc.sync.dma_start(out=na_sb[:, :], in_=na_in[:, :])
            fyo_sb = cst.tile([128, 3 * T], F16)
            nc.sync.dma_start(out=fyo_sb[:, :], in_=fyo_in[:, :])
            idx_sb = cst.tile([128, T * 8], I16)
            nc.sync.dma_start(out=idx_sb[:, :], in_=idx_in[:, :])

            # ---- DRAM buffers
            xc0b = dram.tile([cfg.N, 128], F16)
            nc.sync.dma_start(out=xc0b[:, :], in_=xc0_in[:, :])
            xstate = {}
            cbase = np.concatenate(
                [[0], np.cumsum([r * NCORES for r in cfg.CROWS])]).astype(int)

            def emit_layer(lay):
                nm, D, OCP, TWL = lay['name'], lay['D'], lay['OCP'], lay['tw']
                dmain = lay['dmain']
                gcols = lay['gcols']
                nfull = dmain // 128                     # 12 or 0
                # out-mm K chunks: nfull x 128 + tail TWL
                if nm == 'p0':
                    gsrc = xc0b
                else:
                    gsrc = xstate['cur']

                # layer weights (fp16 direct)
                winT = sbw.tile([7, DMAX], F16, tag="winT")
                nc.sync.dma_start(out=winT[:, 0:D], in_=win_ins[nm][:, :])
                wt = sbw.tile([128, 12 * 512], F16, tag="wt")
                for ci in range(nfull):
                    nc.sync.dma_start(
                        out=wt[:, ci * OCP:ci * OCP + OCP],
                        in_=wout_ins[nm][ci * 128:(ci + 1) * 128, :])
                wtail = sbw.tile([32, 512], F16, tag="wtail")
                nc.sync.dma_start(
                    out=wtail[0:TWL, 0:OCP],
                    in_=wout_ins[nm][nfull * 128:nfull * 128 + TWL, :])
                wbias = sbw.tile([1, 512], F16, tag="wbias")
                nc.sync.dma_start(out=wbias[:, 0:OCP],
                                  in_=wout_ins[nm][D:D + 1, :])

                if nm != 'c2':
                    Xout = dram.tile([cfg.N, HID], F16,
                                     tag="Xbuf", name=f"X_{nm}", bufs=2)
                    oslices = []
                    for k, r in enumerate(cfg.CROWS):
                        t_ = dram.tile([r, HID], F16, tag=f"osl_{k}",
                                       name=f"osl_{nm}_{k}", bufs=1)
                        oslices.append(t_)
                    xstate['cur'] = Xout

                # pipeline state per window
                state = {}

                def st_gather(w):
                    nt = int(twin[w])
                    t0 = int(tbase[w])
                    ka, kb = int(kA[w]), int(kB[w])
                    xst = gst.tile([128, TWMAX, gcols], F16, tag=f"xst{gcols}")
                    if ka:
                        nc.gpsimd.dma_gather(
                            out_ap=xst[:, 0:ka, 0:gcols],
                            in_ap=gsrc[0:cfg.VIEW, :],
                            idxs_ap=idx_sb[:, t0 * 8:(t0 + ka) * 8],
                            num_idxs=ka * 128, num_idxs_reg=ka * 128,
                            elem_size=gcols,
                            queue_num=(2 * w) % 4)
                    if kb:
                        nc.gpsimd.dma_gather(
                            out_ap=xst[:, ka:nt, 0:gcols],
                            in_ap=gsrc[cfg.ABOFS:cfg.ABOFS + cfg.VIEW, :],
                            idxs_ap=idx_sb[:, (t0 + ka) * 8:(t0 + nt) * 8],
                            num_idxs=kb * 128, num_idxs_reg=kb * 128,
                            elem_size=gcols,
                            queue_num=(2 * w + 1) % 4)
                    eaf = eap.tile([7, TWMAX * 128], F16, tag="eaf")
                    nc.sync.dma_start(out=eaf[:, 0:nt * 128],
                                      in_=ea_in[:, t0 * 128:(t0 + nt) * 128])
                    S = Sp.tile([128, TWMAX * 128], F16, tag="S")
                    nc.vector.tensor_tensor(
                        out=S[:, 0:nt * 128].rearrange("p (t n) -> p t n", n=128),
                        in0=iota_f[:, :].unsqueeze(1).broadcast_to([128, nt, 128]),
                        in1=dst_sb[:, t0:t0 + nt].unsqueeze(2).broadcast_to(
                            [128, nt, 128]),
                        op=ISEQ)
                    state[w] = dict(xst=xst, eaf=eaf, S=S)

                def st_scaling_msg(w, scat_w):
                    """Interleave scaling MMs + fused relu-mult for window w
                    with scatter MMs for window scat_w (PE never waits)."""
                    nt = int(twin[w]) if w is not None else 0
                    t0 = int(tbase[w]) if w is not None else 0
                    if w is not None:
                        sw = state[w]
                        msg = msgp.tile([128, TWMAX, DMAX], F16, tag="msg")
                        sw['msg'] = msg
                    # scatter stream for scat_w
                    scat_ops = []
                    if scat_w is not None:
                        ssw = state[scat_w]
                        snt = int(twin[scat_w])
                        agps = ps_ag.tile([128, 2048], F32, tag="agps")
                        ssw['agps'] = agps
                        smsg = ssw['msg']
                        sS = ssw['S']
                        npieces = (3 if dmain else 0) + 1
                        for t in range(snt):
                            for pi in range(npieces):
                                if dmain and pi < 3:
                                    lo, hi = pi * 512, (pi + 1) * 512
                                else:
                                    lo, hi = dmain, dmain + TWL
                                scat_ops.append((t, lo, hi))
                        scat_i = [0]

                        def emit_scat(n=1):
                            for _ in range(n):
                                if scat_i[0] >= len(scat_ops):
                                    return
                                t, lo, hi = scat_ops[scat_i[0]]
                                scat_i[0] += 1
                                nc.tensor.matmul(
                                    agps[:, lo:hi],
                                    sS[:, t * 128:(t + 1) * 128],
                                    smsg[:, t, lo:hi],
                                    start=(t == 0), stop=(t == snt - 1))
                    else:
                        def emit_scat(n=1):
                            return

                    # scaling + fused relu*x for w, interleaved
                    eng_i = [0]
                    for t in range(nt):
                        tg = t0 + t
                        pieces = ([(p * 512, (p + 1) * 512) for p in range(3)]
                                  if dmain else [])
                        pieces.append((dmain, dmain + TWL))
                        for (lo, hi) in pieces:
                            scps = ps_sc.tile([128, 512], F32, tag="scps")
                            nc.tensor.matmul(
                                scps[:, 0:hi - lo],
                                sw['eaf'][:, t * 128:(t + 1) * 128],
                                winT[:, lo:hi], start=True, stop=True)
                            emit_scat(1)
                            # msg = relu(scps) * x_j  (GPSIMD can't touch
                            # PSUM: path A = DVE fused STT; B/C = ACT relu
                            # then DVE-2x / GPSIMD multiply)
                            eng = nc.vector
                            if dmain and hi <= dmain:
                                path = 'ACB'[eng_i[0] % 3]
                                eng_i[0] += 1
                                if path == 'A':
                                    nc.vector.scalar_tensor_tensor(
                                        out=msg[:, t, lo:hi],
                                        in0=scps[:, 0:512],
                                        scalar=0.0, in1=sw['xst'][:, t, 0:512],
                                        op0=MAX, op1=MULT)
                                else:
                                    sc = scp.tile([128, 512], F16, tag="sc")
                                    nc.scalar.activation(sc[:, :],
                                                         scps[:, 0:512], RELU)
                                    meng = nc.vector if path == 'B' else nc.gpsimd
                                    meng.tensor_tensor(
                                        out=msg[:, t, lo:hi], in0=sc[:, :],
                                        in1=sw['xst'][:, t, 0:512], op=MULT)
                            elif nm == 'p0':
                                eng.scalar_tensor_tensor(
                                    out=msg[:, t, 0:24].rearrange(
                                        "p (h i) -> p h i", i=8),
                                    in0=scps[:, 0:24].rearrange(
                                        "p (h i) -> p h i", i=8),
                                    scalar=0.0,
                                    in1=sw['xst'][:, t, 0:8].unsqueeze(1)
                                    .broadcast_to([128, 3, 8]),
                                    op0=MAX, op1=MULT)
                            else:
                                # tail: na (3) [+ fyo (9) before na for c0]
                                if nm == 'c0':
                                    eng.scalar_tensor_tensor(
                                        out=msg[:, t, dmain:dmain + 9].rearrange(
                                            "p (f h) -> p f h", h=3),
                                        in0=scps[:, 0:9].rearrange(
                                            "p (f h) -> p f h", h=3),
                                        scalar=0.0,
                                        in1=fyo_sb[:, 3 * tg:3 * tg + 3]
                                        .unsqueeze(2).broadcast_to([128, 3, 3]),
                                        op0=MAX, op1=MULT)
                                    nlo, ntw = 9, 3
                                else:
                                    nlo, ntw = 0, TWL
                                eng.scalar_tensor_tensor(
                                    out=msg[:, t, dmain + nlo:dmain + nlo + ntw],
                                    in0=scps[:, nlo:nlo + ntw],
                                    scalar=0.0,
                                    in1=na_sb[:, tg:tg + 1].broadcast_to(
                                        [128, ntw]),
                                    op0=MAX, op1=MULT)
                    emit_scat(10000)

                def st_evac(w):
                    ssw = state[w]
                    ags = agsp.tile([128, DMAX], F16, tag="ags")
                    nc.scalar.activation(ags[:, 0:D], ssw['agps'][:, 0:D], COPY)
                    ssw['ags'] = ags

                def st_transpose(w):
                    ssw = state[w]
                    ags = ssw['ags']
                    agT = agtp.tile([128, 12 * 128], F16, tag="agT")
                    agTt = agtp.tile([32, 128], F16, tag="agTt")
                    for gi in range(0, nfull, 8):
                        gn = min(8, nfull - gi)
                        tp = ps_tp.tile([128, 1024], F16, tag="tp")
                        for u in range(gn):
                            ci = gi + u
                            nc.tensor.transpose(
                                tp[:, u * 128:(u + 1) * 128],
                                ags[:, ci * 128:(ci + 1) * 128],
                                ident[:, :])
                        eng = nc.vector if gi == 0 else nc.scalar
                        if eng is nc.vector:
                            nc.vector.tensor_copy(
                                agT[:, gi * 128:(gi + gn) * 128],
                                tp[:, 0:gn * 128])
                        else:
                            nc.scalar.activation(
                                agT[:, gi * 128:(gi + gn) * 128],
                                tp[:, 0:gn * 128], COPY)
                    tp2 = ps_tp.tile([128, 1024], F16, tag="tp")
                    nc.tensor.transpose(
                        tp2[0:TWL, 0:128],
                        ags[:, dmain:dmain + TWL],
                        ident[:, :])
                    nc.vector.tensor_copy(agTt[0:TWL, :], tp2[0:TWL, 0:128])
                    ssw['agT'] = agT
                    ssw['agTt'] = agTt

                def st_outmm(w, wa, k):
                    ssw = state[w]
                    wsz = cfg.WSIZES[w]
                    omf = ps_tp.tile([128, 512], F32, tag="tp")
                    for ci in range(nfull):
                        nc.tensor.matmul(
                            omf[:, 0:OCP],
                            ssw['agT'][:, ci * 128:(ci + 1) * 128],
                            wt[:, ci * OCP:(ci + 1) * OCP],
                            start=(ci == 0), stop=False)
                    nc.tensor.matmul(omf[:, 0:OCP], ssw['agTt'][0:TWL, :],
                                     wtail[0:TWL, 0:OCP],
                                     start=(nfull == 0), stop=False)
                    nc.tensor.matmul(omf[:, 0:OCP], ones[:, :],
                                     wbias[:, 0:OCP], start=False, stop=True)
                    odt = F32 if nm == 'c2' else F16
                    outs = outp.tile([128, 512], odt, tag="outs")
                    nc.scalar.activation(outs[:, 0:OCP], omf[:, 0:OCP], TANH)
                    if nm == 'c2':
                        nc.sync.dma_start(
                            out=out_fin[w * 128:w * 128 + wsz, :],
                            in_=outs[0:wsz, 0:cfg.OUT])
                    else:
                        outr = outp.tile([128, 512], F16, tag="outr")
                        nc.vector.tensor_tensor(out=outr[:, 0:OCP],
                                                in0=outs[:, 0:OCP],
                                                in1=zeros[:, 0:OCP], op=MAX)
                        ro = sum(cfg.WSIZES[wa:w])
                        nc.sync.dma_start(
                            out=oslices[k][ro:ro + wsz, :],
                            in_=outr[0:wsz, 0:HID])
                    del state[w]

                # window -> chunk lookup
                w_chunk = {}
                for k, (wa, wb) in enumerate(cfg.WCHUNKS):
                    for w in range(wa, wb):
                        w_chunk[w] = (k, wa, wb)

                for it in range(NW + 4):
                    if it < NW:
                        st_gather(it)
                    w1 = it - 1 if 0 <= it - 1 < NW else None
                    w2 = it - 2 if 0 <= it - 2 < NW else None
                    if w1 is not None or w2 is not None:
                        st_scaling_msg(w1, w2)
                    if w2 is not None:
                        st_evac(w2)
                    if 0 <= it - 3 < NW:
                        st_transpose(it - 3)
                    if 0 <= it - 4 < NW:
                        w4 = it - 4
                        k, wa, wb = w_chunk[w4]
                        st_outmm(w4, wa, k)
                        # AllGather when the chunk's last window retires
                        if nm != 'c2' and w4 == wb - 1:
                            r = cfg.CROWS[k]
                            nc.gpsimd.collective_compute(
                                "AllGather", mybir.AluOpType.bypass,
                                replica_groups=[list(range(NCORES))],
                                ins=[oslices[k][:, :]],
                                outs=[Xout[cbase[k]:cbase[k] + NCORES * r, :]])

            for lay in cfg.LAYERS:
                emit_layer(lay)
    nc.compile()
    return nc


def _run(inputs, trace=False):
    cfg = Cfg()
    struct, per_core, wts, xc0, asm = _preprocess(cfg, inputs)
    nc = _build(cfg, struct)
    in_maps = []
    for c in range(NCORES):
        im = dict(per_core[c])
        im['xc0_in'] = xc0
        for k, v in wts.items():
            im[k] = v
        in_maps.append(im)
    res = run_bass_kernel_spmd(nc, in_maps, list(range(NCORES)), trace=trace)
    out = np.zeros((cfg.N, cfg.OUT), np.float32)
    for c in range(NCORES):
        sl = res.results[c]['out_final']
        sel = asm['node_core'] == c
        out[sel] = sl[asm['node_row'][sel]]
    return out, res


def kernel(**inputs):
    return _run(inputs, trace=False)[0]


# revision 15
# speedup vs baseline: 1.2474x; 1.2474x over previous
"""Trainium2 Bass kernel for nn_CFDFVnewGCN (6-layer FVnewConv GNN).

Strategy: shard destination nodes (and their incoming edges) across 8 cores.
Nodes are permuted/degree-balanced into 49 windows of <=128 nodes per core.
fp16 datapath end-to-end (PSUM accumulation stays f32).  Per 128-edge tile:
scaling matmul (edge_attr stationary, bias folded as 8th K-row) into PSUM,
fused relu*x_j via scalar_tensor_tensor (PSUM -> fp16 msg, split between DVE
and GPSIMD), one-hot scatter matmul accumulating aggr[window, D] in PSUM.
Per window: PE transposes of aggr + output matmul (bias via const ones row),
tanh (ACT) + relu (DVE).  d-layout: 3 planes of 512 gathered x cols
(d = h*512 + i), then a small tail (na, and fyo for c0) so the scatter PSUM
is exactly 4 banks.  Software pipeline: at iteration `it` we emit
gather(it) / scaling+msg(it-1) interleaved with scatter(it-2) /
transposes(it-3) / out-matmul(it-4), keeping the PE stream dependency-free
and continuously busy (full 2.4 GHz p-state).  x replicated via 4 chunked
AllGathers per layer (overlapping compute) into a Shared DRAM buffer.
Gather uses dma_gather with two offset views of the x buffer (rows 0:32768
and 17232:50000) so indices fit int16.
"""
import sys
import numpy as np

for _p in ('/opt/trn_rl_repo', '/root/.axon_site/_ro/trn_rl_repo'):
    if _p not in sys.path:
        sys.path.insert(0, _p)

import concourse.bacc as bacc
import concourse.mybir as mybir
import concourse.tile as tile
from concourse.bass_utils import run_bass_kernel_spmd

F32 = mybir.dt.float32
F16 = mybir.dt.float16
I16 = mybir.dt.int16
I32 = mybir.dt.int32
COPY = mybir.ActivationFunctionType.Copy
RELU = mybir.ActivationFunctionType.Relu
TANH = mybir.ActivationFunctionType.Tanh
MULT = mybir.AluOpType.mult
MAX = mybir.AluOpType.max
ISEQ = mybir.AluOpType.is_equal

NCORES = 8
G = 512          # gathered x columns (one plane width)
DMAIN = 3 * G    # 1536


class Cfg:
    def __init__(self, n_nodes=50000, n_edges=200000, hid=512, hs=3, ea=6,
                 out=3, n_ag_chunks=4):
        self.N = n_nodes
        self.E = n_edges
        self.HID = hid
        self.HS = hs
        self.EA = ea
        self.OUT = out
        self.NPC = self.N // NCORES              # nodes per core
        self.NWIN = (self.NPC + 127) // 128      # windows per core
        self.WSIZES = [128] * (self.NWIN - 1) + [self.NPC - 128 * (self.NWIN - 1)]
        # A/B view split of the x buffer rows (int16 gather index range)
        self.VIEW = min(32768, self.N)
        self.ABOFS = max(0, self.N - self.VIEW)
        # AllGather chunking: split windows into n_ag_chunks groups
        if n_ag_chunks == 4 and self.NWIN == 49:
            bounds = [0, 20, 35, 45, 49]
        else:
            k = min(n_ag_chunks, self.NWIN)
            bounds = [round(i * self.NWIN / k) for i in range(k + 1)]
        self.WCHUNKS = [(bounds[i], bounds[i + 1]) for i in range(len(bounds) - 1)]
        self.CROWS = [sum(self.WSIZES[a:b]) for a, b in self.WCHUNKS]
        # layer table
        #  p0: ic=7  ([x5,sdf,na] + pad), gathered from xc0 (16 fp16 cols)
        #  others: 3 planes of 512 x-cols + tail (na=3; c0 adds fyo=9)
        self.LAYERS = []
        for name in ['p0', 'p1', 'p2', 'c0', 'c1', 'c2']:
            if name == 'p0':
                lay = dict(name=name, oc=hid, dmain=0, tw=24, gcols=128,
                           relu=True)
            else:
                oc = out if name == 'c2' else hid
                tw = 12 if name == 'c0' else 4
                lay = dict(name=name, oc=oc, dmain=DMAIN, tw=tw, gcols=G,
                           relu=(name != 'c2'))
            lay['D'] = lay['dmain'] + lay['tw']
            lay['OCP'] = lay['oc'] + (-lay['oc']) % 4
            self.LAYERS.append(lay)


def _col2orig(cfg, lay):
    """Map new d-layout column -> original scaling index j=i*HS+h, -1=pad."""
    HS = cfg.HS
    m = np.full(lay['D'], -1, np.int64)
    nm = lay['name']
    if nm == 'p0':
        # d = h*8 + i, i<7 ([x5, sdf, na])
        for h in range(HS):
            for i in range(7):
                m[h * 8 + i] = i * HS + h
    else:
        # ic layout in reference xc: c0: [fyo3, x512, na1]; else [x512, na1]
        xoff = 3 if nm == 'c0' else 0
        for h in range(HS):
            for i in range(G):
                m[h * G + i] = (xoff + i) * HS + h
        t0 = DMAIN
        if nm == 'c0':
            for f in range(3):
                for h in range(HS):
                    m[t0 + 3 * f + h] = f * HS + h        # fyo
            for h in range(HS):
                m[t0 + 9 + h] = (3 + G) * HS + h          # na
        else:
            for h in range(HS):
                m[t0 + h] = G * HS + h                    # na
    return m


def _balance(items_deg, caps):
    """Greedy: assign items (sorted by degree desc) to bins with capacity,
    minimizing max degree sum. Returns bin index per item."""
    order = np.argsort(-items_deg, kind='stable')
    nbins = len(caps)
    load = np.zeros(nbins)
    cnt = np.zeros(nbins, np.int64)
    out = np.zeros(len(items_deg), np.int64)
    import heapq
    heap = [(0.0, b) for b in range(nbins)]
    heapq.heapify(heap)
    for it in order:
        while True:
            l, b = heapq.heappop(heap)
            if cnt[b] < caps[b]:
                break
        out[it] = b
        cnt[b] += 1
        load[b] += items_deg[it]
        if cnt[b] < caps[b]:
            heapq.heappush(heap, (load[b], b))
    return out


def _preprocess(cfg, inputs):
    N, E, HS = cfg.N, cfg.E, cfg.HS
    ei = np.asarray(inputs['edge_index'])
    src = ei[0].astype(np.int64)
    dst = ei[1].astype(np.int64)
    deg = np.bincount(dst, minlength=N).astype(np.float64)

    node_core = _balance(deg, [cfg.NPC] * NCORES)
    node_win = np.zeros(N, np.int64)
    node_slot = np.zeros(N, np.int64)
    for c in range(NCORES):
        nodes = np.where(node_core == c)[0]
        w = _balance(deg[nodes], cfg.WSIZES)
        node_win[nodes] = w
        for wi in range(cfg.NWIN):
            sel = nodes[w == wi]
            node_slot[sel] = np.arange(len(sel))

    # within-core row and global x row (AG chunk-major, rank-interleaved)
    node_row = node_win * 128 + node_slot
    cbase = np.concatenate([[0], np.cumsum([r * NCORES for r in cfg.CROWS])])
    wchunk = np.zeros(cfg.NWIN, np.int64)
    wofs = np.zeros(cfg.NWIN, np.int64)
    for k, (a, b) in enumerate(cfg.WCHUNKS):
        for w in range(a, b):
            wchunk[w] = k
            wofs[w] = sum(cfg.WSIZES[a:w])
    k_of = wchunk[node_win]
    xrow = (cbase[k_of] + node_core * np.array(cfg.CROWS)[k_of]
            + wofs[node_win] + node_slot)
    xrow_src = xrow[src]

    # edge buckets per (core, window)
    ec = node_core[dst]
    ew = node_win[dst]
    # forced side by xrow of src
    fA = xrow_src < cfg.ABOFS
    fB = xrow_src >= cfg.VIEW

    # per-window global tile structure (max over cores)
    kA = np.zeros(cfg.NWIN, np.int64)
    kB = np.zeros(cfg.NWIN, np.int64)
    tw = np.zeros(cfg.NWIN, np.int64)
    cntA = np.zeros((NCORES, cfg.NWIN), np.int64)
    cntB = np.zeros((NCORES, cfg.NWIN), np.int64)
    cntT = np.zeros((NCORES, cfg.NWIN), np.int64)
    np.add.at(cntA, (ec[fA], ew[fA]), 1)
    np.add.at(cntB, (ec[fB], ew[fB]), 1)
    np.add.at(cntT, (ec, ew), 1)
    for w in range(cfg.NWIN):
        ka = int(np.ceil(cntA[:, w].max() / 128))
        kb = int(np.ceil(cntB[:, w].max() / 128))
        t = max(ka + kb, int(np.ceil(cntT[:, w].max() / 128)), 1)
        kA[w] = ka
        kB[w] = t - ka
        tw[w] = t
        assert kB[w] >= kb
    tbase = np.concatenate([[0], np.cumsum(tw)])
    T = int(tbase[-1])

    ea_np = np.asarray(inputs['edge_attr'], np.float32)
    na_np = np.asarray(inputs['node_attr'], np.float32).reshape(-1)
    fyo_np = np.asarray(inputs['fine_y_orig'], np.float32)

    per_core = []
    for c in range(NCORES):
        ea_s = np.zeros((7, T * 128), np.float16)
        idx_s = np.zeros((16, T * 8), np.int16)
        dst_s = np.full((128, T), 999.0, np.float16)
        na_s = np.zeros((128, T), np.float16)
        fyo_s = np.zeros((128, 3 * T), np.float16)
        for w in range(cfg.NWIN):
            eidx = np.where((ec == c) & (ew == w))[0]
            if len(eidx):
                a_e = eidx[fA[eidx]]
                m_e = eidx[~fA[eidx] & ~fB[eidx]]
                b_e = eidx[fB[eidx]]
                capA = int(kA[w]) * 128
                take = min(len(m_e), capA - len(a_e))
                A = np.concatenate([a_e, m_e[:take]])
                B = np.concatenate([b_e, m_e[take:]])
            else:
                A = B = np.array([], np.int64)
            assert len(A) <= kA[w] * 128 and len(B) <= kB[w] * 128, (w, len(A), len(B))
            for side, edges, ktiles, t0 in (
                    (0, A, int(kA[w]), int(tbase[w])),
                    (1, B, int(kB[w]), int(tbase[w] + kA[w]))):
                nslots = ktiles * 128
                if nslots == 0:
                    continue
                iv = np.zeros(nslots, np.int64)
                iv[:len(edges)] = xrow_src[edges] - (0 if side == 0 else cfg.ABOFS)
                assert iv.min() >= 0 and iv.max() < 32768, (iv.min(), iv.max())
                jj = np.arange(nslots)
                tt = t0 + jj // 128
                pp = jj % 128
                idx_s[jj % 16, t0 * 8 + jj // 16] = iv.astype(np.int16)
                if len(edges):
                    e_jj = jj[:len(edges)]
                    e_tt = tt[:len(edges)]
                    e_pp = pp[:len(edges)]
                    ea_s[0:6, e_tt * 128 + e_pp] = ea_np[edges].T
                    ea_s[6, e_tt * 128 + e_pp] = 1.0
                    dst_s[e_pp, e_tt] = node_slot[dst[edges]]
                    na_s[e_pp, e_tt] = na_np[src[edges]]
                    fyo_s[e_pp.repeat(3), (e_tt * 3).repeat(3)
                          + np.tile([0, 1, 2], len(edges))] = fyo_np[src[edges]].ravel()
        per_core.append(dict(ea_s=ea_s, idx_s=np.tile(idx_s, (8, 1)),
                             dst_s=dst_s, na_s=na_s, fyo_s=fyo_s))

    # xc0 buffer: [N, 16] fp16 in x-row order: cols [x(5), sdf, na, 0...]
    x_np = np.asarray(inputs['x'], np.float32)
    sdf_np = np.asarray(inputs['sdf'], np.float32)
    xc0 = np.zeros((N, 128), np.float16)
    xc0[xrow, 0:x_np.shape[1]] = x_np
    xc0[xrow, x_np.shape[1]] = sdf_np[:, 0]
    xc0[xrow, x_np.shape[1] + 1] = na_np

    # weights per layer (fp16, reordered to kernel d-layout)
    wts = {}
    for lay in cfg.LAYERS:
        nm = lay['name']
        win = np.asarray(inputs[f'win_{nm}'], np.float32)
        bin_ = np.asarray(inputs[f'bin_{nm}'], np.float32)
        wout = np.asarray(inputs[f'wout_{nm}'], np.float32)
        bout = np.asarray(inputs[f'bout_{nm}'], np.float32)
        m = _col2orig(cfg, lay)
        D, OCP = lay['D'], lay['OCP']
        winT = np.zeros((7, D), np.float16)
        sel = m >= 0
        winT[0:cfg.EA, sel] = win[m[sel]].T
        winT[6, sel] = bin_[m[sel]]
        woutT = np.zeros((D + 1, OCP), np.float16)
        woutT[np.where(sel)[0], 0:lay['oc']] = wout[:, m[sel]].T
        woutT[D, 0:lay['oc']] = bout
        wts[f'winT_{nm}'] = winT
        wts[f'woutT_{nm}'] = woutT

    struct = dict(kA=kA, kB=kB, tw=tw, tbase=tbase, T=T,
                  TWMAX=int(tw.max()))
    asm = dict(node_core=node_core, node_row=node_row)
    return struct, per_core, wts, xc0, asm


def _build(cfg, struct):
    kA, kB, twin, tbase, T = (struct['kA'], struct['kB'], struct['tw'],
                              struct['tbase'], struct['T'])
    TWMAX = struct['TWMAX']
    HID = cfg.HID
    NW = cfg.NWIN
    DMAX = max(l['D'] for l in cfg.LAYERS)

    nc = bacc.Bacc("TRN2", target_bir_lowering=False, debug=False,
                   enable_asserts=True, num_devices=NCORES,
                   num_swdge_queues=4)
    ea_in = nc.dram_tensor("ea_s", [7, T * 128], F16, kind="ExternalInput").ap()
    idx_in = nc.dram_tensor("idx_s", [128, T * 8], I16, kind="ExternalInput").ap()
    dst_in = nc.dram_tensor("dst_s", [128, T], F16, kind="ExternalInput").ap()
    na_in = nc.dram_tensor("na_s", [128, T], F16, kind="ExternalInput").ap()
    fyo_in = nc.dram_tensor("fyo_s", [128, 3 * T], F16, kind="ExternalInput").ap()
    xc0_in = nc.dram_tensor("xc0_in", [cfg.N, 128], F16, kind="ExternalInput").ap()
    win_ins = {}
    wout_ins = {}
    for lay in cfg.LAYERS:
        nm = lay['name']
        win_ins[nm] = nc.dram_tensor(f"winT_{nm}", [7, lay['D']], F16,
                                     kind="ExternalInput").ap()
        wout_ins[nm] = nc.dram_tensor(f"woutT_{nm}", [lay['D'] + 1, lay['OCP']],
                                      F16, kind="ExternalInput").ap()
    out_fin = nc.dram_tensor("out_final", [cfg.NPC, cfg.OUT], F32,
                             kind="ExternalOutput").ap()

    with tile.TileContext(nc) as tc:
        with (
            tc.tile_pool(name="cst", bufs=1) as cst,
            tc.tile_pool(name="sbw", bufs=2) as sbw,
            tc.tile_pool(name="gst", bufs=3) as gst,
            tc.tile_pool(name="eap", bufs=3) as eap,
            tc.tile_pool(name="msgp", bufs=2) as msgp,
            tc.tile_pool(name="scp", bufs=3) as scp,
            tc.tile_pool(name="Sp", bufs=3) as Sp,
            tc.tile_pool(name="agsp", bufs=2) as agsp,
            tc.tile_pool(name="agtp", bufs=2) as agtp,
            tc.tile_pool(name="outp", bufs=2) as outp,
            tc.tile_pool(name="ps_sc", bufs=3, space="PSUM") as ps_sc,
            tc.tile_pool(name="ps_ag", bufs=1, space="PSUM") as ps_ag,
            tc.tile_pool(name="ps_tp", bufs=1, space="PSUM") as ps_tp,
            tc.tile_pool(name="dram", bufs=1, space="DRAM") as dram,
        ):
            # ---- constants
            iota_i = cst.tile([128, 128], I32)
            nc.gpsimd.iota(iota_i[:, :], pattern=[[1, 128]], base=0,
                           channel_multiplier=0)
            iota_f = cst.tile([128, 128], F16)
            nc.vector.tensor_copy(iota_f[:, :], iota_i[:, :])
            iota_p = cst.tile([128, 1], I32)
            nc.gpsimd.iota(iota_p[:, :], pattern=[[1, 1]], base=0,
                           channel_multiplier=1)
            iota_pf = cst.tile([128, 1], F32)
            nc.vector.tensor_copy(iota_pf[:, :], iota_p[:, :])
            identf = cst.tile([128, 128], F32)
            nc.vector.tensor_scalar(out=identf[:, :], in0=iota_f[:, :],
                                    scalar1=iota_pf[:, :], scalar2=None,
                                    op0=ISEQ)
            ident = cst.tile([128, 128], F16)
            nc.vector.tensor_copy(ident[:, :], identf[:, :])
            ones_i = cst.tile([1, 128], I32)
            nc.gpsimd.iota(ones_i[:, :], pattern=[[0, 128]], base=1,
                           channel_multiplier=0)
            ones = cst.tile([1, 128], F16)
            nc.vector.tensor_copy(ones[:, :], ones_i[:, :])
            zeros = cst.tile([128, 512], F16)
            nc.vector.memset(zeros[:, :], 0)

            # ---- static per-slot data (resident)
            dst_sb = cst.tile([128, T], F16)
            nc.sync.dma_start(out=dst_sb[:, :], in_=dst_in[:, :])
            na_sb = cst.tile([128, T], F16)
            nc.sync.dma_start(out=na_sb[:, :], in_=na_in[:, :])
            fyo_sb = cst.tile([128, 3 * T], F16)
            nc.sync.dma_start(out=fyo_sb[:, :], in_=fyo_in[:, :])
            idx_sb = cst.tile([128, T * 8], I16)
            nc.sync.dma_start(out=idx_sb[:, :], in_=idx_in[:, :])

            # ---- DRAM buffers
            xc0b = dram.tile([cfg.N, 128], F16)
            nc.sync.dma_start(out=xc0b[:, :], in_=xc0_in[:, :])
            xstate = {}
            cbase = np.concatenate(
                [[0], np.cumsum([r * NCORES for r in cfg.CROWS])]).astype(int)

            def emit_layer(lay):
                nm, D, OCP, TWL = lay['name'], lay['D'], lay['OCP'], lay['tw']
                dmain = lay['dmain']
                gcols = lay['gcols']
                nfull = dmain // 128                     # 12 or 0
                # out-mm K chunks: nfull x 128 + tail TWL
                if nm == 'p0':
                    gsrc = xc0b
                else:
                    gsrc = xstate['cur']

                # layer weights (fp16 direct)
                winT = sbw.tile([7, DMAX], F16, tag="winT")
                nc.sync.dma_start(out=winT[:, 0:D], in_=win_ins[nm][:, :])
                wt = sbw.tile([128, 12 * 512], F16, tag="wt")
                for ci in range(nfull):
                    nc.sync.dma_start(
                        out=wt[:, ci * OCP:ci * OCP + OCP],
                        in_=wout_ins[nm][ci * 128:(ci + 1) * 128, :])
                wtail = sbw.tile([32, 512], F16, tag="wtail")
                nc.sync.dma_start(
                    out=wtail[0:TWL, 0:OCP],
                    in_=wout_ins[nm][nfull * 128:nfull * 128 + TWL, :])
                wbias = sbw.tile([1, 512], F16, tag="wbias")
                nc.sync.dma_start(out=wbias[:, 0:OCP],
                                  in_=wout_ins[nm][D:D + 1, :])

                if nm != 'c2':
                    Xout = dram.tile([cfg.N, HID], F16,
                                     tag="Xbuf", name=f"X_{nm}", bufs=2)
                    oslices = []
                    for k, r in enumerate(cfg.CROWS):
                        t_ = dram.tile([r, HID], F16, tag=f"osl_{k}",
                                       name=f"osl_{nm}_{k}", bufs=1)
                        oslices.append(t_)
                    xstate['cur'] = Xout

                # pipeline state per window
                state = {}

                def st_gather(w):
                    nt = int(twin[w])
                    t0 = int(tbase[w])
                    ka, kb = int(kA[w]), int(kB[w])
                    xst = gst.tile([128, TWMAX, gcols], F16, tag=f"xst{gcols}")
                    if ka:
                        nc.gpsimd.dma_gather(
                            out_ap=xst[:, 0:ka, 0:gcols],
                            in_ap=gsrc[0:cfg.VIEW, :],
                            idxs_ap=idx_sb[:, t0 * 8:(t0 + ka) * 8],
                            num_idxs=ka * 128, num_idxs_reg=ka * 128,
                            elem_size=gcols,
                            queue_num=(2 * w) % 4)
                    if kb:
                        nc.gpsimd.dma_gather(
                            out_ap=xst[:, ka:nt, 0:gcols],
                            in_ap=gsrc[cfg.ABOFS:cfg.ABOFS + cfg.VIEW, :],
                            idxs_ap=idx_sb[:, (t0 + ka) * 8:(t0 + nt) * 8],
                            num_idxs=kb * 128, num_idxs_reg=kb * 128,
                            elem_size=gcols,
                            queue_num=(2 * w + 1) % 4)
                    eaf = eap.tile([7, TWMAX * 128], F16, tag="eaf")
                    nc.sync.dma_start(out=eaf[:, 0:nt * 128],
                                      in_=ea_in[:, t0 * 128:(t0 + nt) * 128])
                    S = Sp.tile([128, TWMAX * 128], F16, tag="S")
                    nc.vector.tensor_tensor(
                        out=S[:, 0:nt * 128].rearrange("p (t n) -> p t n", n=128),
                        in0=iota_f[:, :].unsqueeze(1).broadcast_to([128, nt, 128]),
                        in1=dst_sb[:, t0:t0 + nt].unsqueeze(2).broadcast_to(
                            [128, nt, 128]),
                        op=ISEQ)
                    state[w] = dict(xst=xst, eaf=eaf, S=S)

                def st_scaling_msg(w, scat_w):
                    """Interleave scaling MMs + fused relu-mult for window w
                    with scatter MMs for window scat_w (PE never waits)."""
                    nt = int(twin[w]) if w is not None else 0
                    t0 = int(tbase[w]) if w is not None else 0
                    if w is not None:
                        sw = state[w]
                        msg = msgp.tile([128, TWMAX, DMAX], F16, tag="msg")
                        sw['msg'] = msg
                    # scatter stream for scat_w
                    scat_ops = []
                    if scat_w is not None:
                        ssw = state[scat_w]
                        snt = int(twin[scat_w])
                        agps = ps_ag.tile([128, 2048], F32, tag="agps")
                        ssw['agps'] = agps
                        smsg = ssw['msg']
                        sS = ssw['S']
                        npieces = (3 if dmain else 0) + 1
                        for t in range(snt):
                            for pi in range(npieces):
                                if dmain and pi < 3:
                                    lo, hi = pi * 512, (pi + 1) * 512
                                else:
                                    lo, hi = dmain, dmain + TWL
                                scat_ops.append((t, lo, hi))
                        scat_i = [0]

                        def emit_scat(n=1):
                            for _ in range(n):
                                if scat_i[0] >= len(scat_ops):
                                    return
                                t, lo, hi = scat_ops[scat_i[0]]
                                scat_i[0] += 1
                                nc.tensor.matmul(
                                    agps[:, lo:hi],
                                    sS[:, t * 128:(t + 1) * 128],
                                    smsg[:, t, lo:hi],
                                    start=(t == 0), stop=(t == snt - 1))
                    else:
                        def emit_scat(n=1):
                            return

                    # scaling + fused relu*x for w, interleaved
                    eng_i = [0]
                    for t in range(nt):
                        tg = t0 + t
                        pieces = ([(p * 512, (p + 1) * 512) for p in range(3)]
                                  if dmain else [])
                        pieces.append((dmain, dmain + TWL))
                        for (lo, hi) in pieces:
                            scps = ps_sc.tile([128, 512], F32, tag="scps")
                            nc.tensor.matmul(
                                scps[:, 0:hi - lo],
                                sw['eaf'][:, t * 128:(t + 1) * 128],
                                winT[:, lo:hi], start=True, stop=True)
                            emit_scat(1)
                            # msg = relu(scps) * x_j  (GPSIMD can't touch
                            # PSUM: path A = DVE fused STT; B/C = ACT relu
                            # then DVE-2x / GPSIMD multiply)
                            eng = nc.vector
                            if dmain and hi <= dmain:
                                path = 'ACB'[eng_i[0] % 3]
                                eng_i[0] += 1
                                if path == 'A':
                                    nc.vector.scalar_tensor_tensor(
                                        out=msg[:, t, lo:hi],
                                        in0=scps[:, 0:512],
                                        scalar=0.0, in1=sw['xst'][:, t, 0:512],
                                        op0=MAX, op1=MULT)
                                else:
                                    sc = scp.tile([128, 512], F16, tag="sc")
                                    nc.scalar.activation(sc[:, :],
                                                         scps[:, 0:512], RELU)
                                    meng = nc.vector if path == 'B' else nc.gpsimd
                                    meng.tensor_tensor(
                                        out=msg[:, t, lo:hi], in0=sc[:, :],
                                        in1=sw['xst'][:, t, 0:512], op=MULT)
                            elif nm == 'p0':
                                eng.scalar_tensor_tensor(
                                    out=msg[:, t, 0:24].rearrange(
                                        "p (h i) -> p h i", i=8),
                                    in0=scps[:, 0:24].rearrange(
                                        "p (h i) -> p h i", i=8),
                                    scalar=0.0,
                                    in1=sw['xst'][:, t, 0:8].unsqueeze(1)
                                    .broadcast_to([128, 3, 8]),
                                    op0=MAX, op1=MULT)
                            else:
                                # tail: na (3) [+ fyo (9) before na for c0]
                                if nm == 'c0':
                                    eng.scalar_tensor_tensor(
                                        out=msg[:, t, dmain:dmain + 9].rearrange(
                                            "p (f h) -> p f h", h=3),
                                        in0=scps[:, 0:9].rearrange(
                                            "p (f h) -> p f h", h=3),
                                        scalar=0.0,
                                        in1=fyo_sb[:, 3 * tg:3 * tg + 3]
                                        .unsqueeze(2).broadcast_to([128, 3, 3]),
                                        op0=MAX, op1=MULT)
                                    nlo, ntw = 9, 3
                                else:
                                    nlo, ntw = 0, TWL
                                eng.scalar_tensor_tensor(
                                    out=msg[:, t, dmain + nlo:dmain + nlo + ntw],
                                    in0=scps[:, nlo:nlo + ntw],
                                    scalar=0.0,
                                    in1=na_sb[:, tg:tg + 1].broadcast_to(
                                        [128, ntw]),
                                    op0=MAX, op1=MULT)
                    emit_scat(10000)

                def st_evac(w):
                    ssw = state[w]
                    ags = agsp.tile([128, DMAX], F16, tag="ags")
                    nc.scalar.activation(ags[:, 0:D], ssw['agps'][:, 0:D], COPY)
                    ssw['ags'] = ags

                def st_transpose(w):
                    ssw = state[w]
                    ags = ssw['ags']
                    agT = agtp.tile([128, 12 * 128], F16, tag="agT")
                    agTt = agtp.tile([32, 128], F16, tag="agTt")
                    for gi in range(0, nfull, 8):
                        gn = min(8, nfull - gi)
                        tp = ps_tp.tile([128, 1024], F16, tag="tp")
                        for u in range(gn):
                            ci = gi + u
                            nc.tensor.transpose(
                                tp[:, u * 128:(u + 1) * 128],
                                ags[:, ci * 128:(ci + 1) * 128],
                                ident[:, :])
                        eng = nc.vector if gi == 0 else nc.scalar
                        if eng is nc.vector:
                            nc.vector.tensor_copy(
                                agT[:, gi * 128:(gi + gn) * 128],
                                tp[:, 0:gn * 128])
                        else:
                            nc.scalar.activation(
                                agT[:, gi * 128:(gi + gn) * 128],
                                tp[:, 0:gn * 128], COPY)
                    tp2 = ps_tp.tile([128, 1024], F16, tag="tp")
                    nc.tensor.transpose(
                        tp2[0:TWL, 0:128],
                        ags[:, dmain:dmain + TWL],
                        ident[:, :])
                    nc.vector.tensor_copy(agTt[0:TWL, :], tp2[0:TWL, 0:128])
                    ssw['agT'] = agT
                    ssw['agTt'] = agTt

                def st_outmm(w, wa, k):
                    ssw = state[w]
                    wsz = cfg.WSIZES[w]
                    omf = ps_tp.tile([128, 512], F32, tag="tp")
                    for ci in range(nfull):
                        nc.tensor.matmul(
                            omf[:, 0:OCP],
                            ssw['agT'][:, ci * 128:(ci + 1) * 128],
                            wt[:, ci * OCP:(ci + 1) * OCP],
                            start=(ci == 0), stop=False)
                    nc.tensor.matmul(omf[:, 0:OCP], ssw['agTt'][0:TWL, :],
                                     wtail[0:TWL, 0:OCP],
                                     start=(nfull == 0), stop=False)
                    nc.tensor.matmul(omf[:, 0:OCP], ones[:, :],
                                     wbias[:, 0:OCP], start=False, stop=True)
                    odt = F32 if nm == 'c2' else F16
                    outs = outp.tile([128, 512], odt, tag="outs")
                    nc.scalar.activation(outs[:, 0:OCP], omf[:, 0:OCP], TANH)
                    if nm == 'c2':
                        nc.sync.dma_start(
                            out=out_fin[w * 128:w * 128 + wsz, :],
                            in_=outs[0:wsz, 0:cfg.OUT])
                    else:
                        outr = outp.tile([128, 512], F16, tag="outr")
                        nc.vector.tensor_tensor(out=outr[:, 0:OCP],
                                                in0=outs[:, 0:OCP],
                                                in1=zeros[:, 0:OCP], op=MAX)
                        ro = sum(cfg.WSIZES[wa:w])
                        nc.sync.dma_start(
                            out=oslices[k][ro:ro + wsz, :],
                            in_=outr[0:wsz, 0:HID])
                    del state[w]

                # window -> chunk lookup
                w_chunk = {}
                for k, (wa, wb) in enumerate(cfg.WCHUNKS):
                    for w in range(wa, wb):
                        w_chunk[w] = (k, wa, wb)

                for it in range(NW + 4):
                    if 0 <= it - 3 < NW:
                        st_transpose(it - 3)
                    if 0 <= it - 4 < NW:
                        w4 = it - 4
                        k, wa, wb = w_chunk[w4]
                        st_outmm(w4, wa, k)
                        # AllGather when the chunk's last window retires
                        if nm != 'c2' and w4 == wb - 1:
                            r = cfg.CROWS[k]
                            nc.gpsimd.collective_compute(
                                "AllGather", mybir.AluOpType.bypass,
                                replica_groups=[list(range(NCORES))],
                                ins=[oslices[k][:, :]],
                                outs=[Xout[cbase[k]:cbase[k] + NCORES * r, :]])
                    w1 = it - 1 if 0 <= it - 1 < NW else None
                    w2 = it - 2 if 0 <= it - 2 < NW else None
                    if w1 is not None or w2 is not None:
                        st_scaling_msg(w1, w2)
                    if w2 is not None:
                        st_evac(w2)
                    if it < NW:
                        st_gather(it)

            for lay in cfg.LAYERS:
                emit_layer(lay)
    nc.compile()
    return nc


def _run(inputs, trace=False):
    cfg = Cfg()
    struct, per_core, wts, xc0, asm = _preprocess(cfg, inputs)
    nc = _build(cfg, struct)
    in_maps = []
    for c in range(NCORES):
        im = dict(per_core[c])
        im['xc0_in'] = xc0
        for k, v in wts.items():
            im[k] = v
        in_maps.append(im)
    res = run_bass_kernel_spmd(nc, in_maps, list(range(NCORES)), trace=trace)
    out = np.zeros((cfg.N, cfg.OUT), np.float32)
    for c in range(NCORES):
        sl = res.results[c]['out_final']
        sel = asm['node_core'] == c
        out[sel] = sl[asm['node_row'][sel]]
    return out, res


def kernel(**inputs):
    return _run(inputs, trace=False)[0]


# revision 16
# speedup vs baseline: 1.2534x; 1.0048x over previous
"""Trainium2 Bass kernel for nn_CFDFVnewGCN (6-layer FVnewConv GNN).

Strategy: shard destination nodes (and their incoming edges) across 8 cores.
Nodes are permuted/degree-balanced into 49 windows of <=128 nodes per core.
fp16 datapath end-to-end (PSUM accumulation stays f32).  Per 128-edge tile:
scaling matmul (edge_attr stationary, bias folded as 8th K-row) into PSUM,
fused relu*x_j via scalar_tensor_tensor (PSUM -> fp16 msg, split between DVE
and GPSIMD), one-hot scatter matmul accumulating aggr[window, D] in PSUM.
Per window: PE transposes of aggr + output matmul (bias via const ones row),
tanh (ACT) + relu (DVE).  d-layout: 3 planes of 512 gathered x cols
(d = h*512 + i), then a small tail (na, and fyo for c0) so the scatter PSUM
is exactly 4 banks.  Software pipeline: at iteration `it` we emit
gather(it) / scaling+msg(it-1) interleaved with scatter(it-2) /
transposes(it-3) / out-matmul(it-4), keeping the PE stream dependency-free
and continuously busy (full 2.4 GHz p-state).  x replicated via 4 chunked
AllGathers per layer (overlapping compute) into a Shared DRAM buffer.
Gather uses dma_gather with two offset views of the x buffer (rows 0:32768
and 17232:50000) so indices fit int16.
"""
import sys
import numpy as np

for _p in ('/opt/trn_rl_repo', '/root/.axon_site/_ro/trn_rl_repo'):
    if _p not in sys.path:
        sys.path.insert(0, _p)

import concourse.bacc as bacc
import concourse.mybir as mybir
import concourse.tile as tile
from concourse.bass_utils import run_bass_kernel_spmd

F32 = mybir.dt.float32
F16 = mybir.dt.float16
I16 = mybir.dt.int16
I32 = mybir.dt.int32
COPY = mybir.ActivationFunctionType.Copy
RELU = mybir.ActivationFunctionType.Relu
TANH = mybir.ActivationFunctionType.Tanh
MULT = mybir.AluOpType.mult
MAX = mybir.AluOpType.max
ISEQ = mybir.AluOpType.is_equal

NCORES = 8
G = 512          # gathered x columns (one plane width)
DMAIN = 3 * G    # 1536


class Cfg:
    def __init__(self, n_nodes=50000, n_edges=200000, hid=512, hs=3, ea=6,
                 out=3, n_ag_chunks=4):
        self.N = n_nodes
        self.E = n_edges
        self.HID = hid
        self.HS = hs
        self.EA = ea
        self.OUT = out
        self.NPC = self.N // NCORES              # nodes per core
        self.NWIN = (self.NPC + 127) // 128      # windows per core
        self.WSIZES = [128] * (self.NWIN - 1) + [self.NPC - 128 * (self.NWIN - 1)]
        # A/B view split of the x buffer rows (int16 gather index range)
        self.VIEW = min(32768, self.N)
        self.ABOFS = max(0, self.N - self.VIEW)
        # AllGather chunking: split windows into n_ag_chunks groups
        if n_ag_chunks == 4 and self.NWIN == 49:
            bounds = [0, 20, 35, 45, 49]
        else:
            k = min(n_ag_chunks, self.NWIN)
            bounds = [round(i * self.NWIN / k) for i in range(k + 1)]
        self.WCHUNKS = [(bounds[i], bounds[i + 1]) for i in range(len(bounds) - 1)]
        self.CROWS = [sum(self.WSIZES[a:b]) for a, b in self.WCHUNKS]
        # layer table
        #  p0: ic=7  ([x5,sdf,na] + pad), gathered from xc0 (16 fp16 cols)
        #  others: 3 planes of 512 x-cols + tail (na=3; c0 adds fyo=9)
        self.LAYERS = []
        for name in ['p0', 'p1', 'p2', 'c0', 'c1', 'c2']:
            if name == 'p0':
                lay = dict(name=name, oc=hid, dmain=0, tw=24, gcols=128,
                           relu=True)
            else:
                oc = out if name == 'c2' else hid
                tw = 12 if name == 'c0' else 4
                lay = dict(name=name, oc=oc, dmain=DMAIN, tw=tw, gcols=G,
                           relu=(name != 'c2'))
            lay['D'] = lay['dmain'] + lay['tw']
            lay['OCP'] = lay['oc'] + (-lay['oc']) % 4
            self.LAYERS.append(lay)


def _col2orig(cfg, lay):
    """Map new d-layout column -> original scaling index j=i*HS+h, -1=pad."""
    HS = cfg.HS
    m = np.full(lay['D'], -1, np.int64)
    nm = lay['name']
    if nm == 'p0':
        # d = h*8 + i, i<7 ([x5, sdf, na])
        for h in range(HS):
            for i in range(7):
                m[h * 8 + i] = i * HS + h
    else:
        # ic layout in reference xc: c0: [fyo3, x512, na1]; else [x512, na1]
        xoff = 3 if nm == 'c0' else 0
        for h in range(HS):
            for i in range(G):
                m[h * G + i] = (xoff + i) * HS + h
        t0 = DMAIN
        if nm == 'c0':
            for f in range(3):
                for h in range(HS):
                    m[t0 + 3 * f + h] = f * HS + h        # fyo
            for h in range(HS):
                m[t0 + 9 + h] = (3 + G) * HS + h          # na
        else:
            for h in range(HS):
                m[t0 + h] = G * HS + h                    # na
    return m


def _balance(items_deg, caps):
    """Greedy: assign items (sorted by degree desc) to bins with capacity,
    minimizing max degree sum. Returns bin index per item."""
    order = np.argsort(-items_deg, kind='stable')
    nbins = len(caps)
    load = np.zeros(nbins)
    cnt = np.zeros(nbins, np.int64)
    out = np.zeros(len(items_deg), np.int64)
    import heapq
    heap = [(0.0, b) for b in range(nbins)]
    heapq.heapify(heap)
    for it in order:
        while True:
            l, b = heapq.heappop(heap)
            if cnt[b] < caps[b]:
                break
        out[it] = b
        cnt[b] += 1
        load[b] += items_deg[it]
        if cnt[b] < caps[b]:
            heapq.heappush(heap, (load[b], b))
    return out


def _preprocess(cfg, inputs):
    N, E, HS = cfg.N, cfg.E, cfg.HS
    ei = np.asarray(inputs['edge_index'])
    src = ei[0].astype(np.int64)
    dst = ei[1].astype(np.int64)
    deg = np.bincount(dst, minlength=N).astype(np.float64)

    node_core = _balance(deg, [cfg.NPC] * NCORES)
    node_win = np.zeros(N, np.int64)
    node_slot = np.zeros(N, np.int64)
    for c in range(NCORES):
        nodes = np.where(node_core == c)[0]
        w = _balance(deg[nodes], cfg.WSIZES)
        node_win[nodes] = w
        for wi in range(cfg.NWIN):
            sel = nodes[w == wi]
            node_slot[sel] = np.arange(len(sel))

    # within-core row and global x row (AG chunk-major, rank-interleaved)
    node_row = node_win * 128 + node_slot
    cbase = np.concatenate([[0], np.cumsum([r * NCORES for r in cfg.CROWS])])
    wchunk = np.zeros(cfg.NWIN, np.int64)
    wofs = np.zeros(cfg.NWIN, np.int64)
    for k, (a, b) in enumerate(cfg.WCHUNKS):
        for w in range(a, b):
            wchunk[w] = k
            wofs[w] = sum(cfg.WSIZES[a:w])
    k_of = wchunk[node_win]
    xrow = (cbase[k_of] + node_core * np.array(cfg.CROWS)[k_of]
            + wofs[node_win] + node_slot)
    xrow_src = xrow[src]

    # edge buckets per (core, window)
    ec = node_core[dst]
    ew = node_win[dst]
    # forced side by xrow of src
    fA = xrow_src < cfg.ABOFS
    fB = xrow_src >= cfg.VIEW

    # per-window global tile structure (max over cores)
    kA = np.zeros(cfg.NWIN, np.int64)
    kB = np.zeros(cfg.NWIN, np.int64)
    tw = np.zeros(cfg.NWIN, np.int64)
    cntA = np.zeros((NCORES, cfg.NWIN), np.int64)
    cntB = np.zeros((NCORES, cfg.NWIN), np.int64)
    cntT = np.zeros((NCORES, cfg.NWIN), np.int64)
    np.add.at(cntA, (ec[fA], ew[fA]), 1)
    np.add.at(cntB, (ec[fB], ew[fB]), 1)
    np.add.at(cntT, (ec, ew), 1)
    for w in range(cfg.NWIN):
        ka = int(np.ceil(cntA[:, w].max() / 128))
        kb = int(np.ceil(cntB[:, w].max() / 128))
        t = max(ka + kb, int(np.ceil(cntT[:, w].max() / 128)), 1)
        kA[w] = ka
        kB[w] = t - ka
        tw[w] = t
        assert kB[w] >= kb
    tbase = np.concatenate([[0], np.cumsum(tw)])
    T = int(tbase[-1])

    ea_np = np.asarray(inputs['edge_attr'], np.float32)
    na_np = np.asarray(inputs['node_attr'], np.float32).reshape(-1)
    fyo_np = np.asarray(inputs['fine_y_orig'], np.float32)

    per_core = []
    for c in range(NCORES):
        ea_s = np.zeros((7, T * 128), np.float16)
        idx_s = np.zeros((16, T * 8), np.int16)
        dst_s = np.full((128, T), 999.0, np.float16)
        na_s = np.zeros((128, T), np.float16)
        fyo_s = np.zeros((128, 3 * T), np.float16)
        for w in range(cfg.NWIN):
            eidx = np.where((ec == c) & (ew == w))[0]
            if len(eidx):
                a_e = eidx[fA[eidx]]
                m_e = eidx[~fA[eidx] & ~fB[eidx]]
                b_e = eidx[fB[eidx]]
                capA = int(kA[w]) * 128
                take = min(len(m_e), capA - len(a_e))
                A = np.concatenate([a_e, m_e[:take]])
                B = np.concatenate([b_e, m_e[take:]])
            else:
                A = B = np.array([], np.int64)
            assert len(A) <= kA[w] * 128 and len(B) <= kB[w] * 128, (w, len(A), len(B))
            for side, edges, ktiles, t0 in (
                    (0, A, int(kA[w]), int(tbase[w])),
                    (1, B, int(kB[w]), int(tbase[w] + kA[w]))):
                nslots = ktiles * 128
                if nslots == 0:
                    continue
                iv = np.zeros(nslots, np.int64)
                iv[:len(edges)] = xrow_src[edges] - (0 if side == 0 else cfg.ABOFS)
                assert iv.min() >= 0 and iv.max() < 32768, (iv.min(), iv.max())
                jj = np.arange(nslots)
                tt = t0 + jj // 128
                pp = jj % 128
                idx_s[jj % 16, t0 * 8 + jj // 16] = iv.astype(np.int16)
                if len(edges):
                    e_jj = jj[:len(edges)]
                    e_tt = tt[:len(edges)]
                    e_pp = pp[:len(edges)]
                    ea_s[0:6, e_tt * 128 + e_pp] = ea_np[edges].T
                    ea_s[6, e_tt * 128 + e_pp] = 1.0
                    dst_s[e_pp, e_tt] = node_slot[dst[edges]]
                    na_s[e_pp, e_tt] = na_np[src[edges]]
                    fyo_s[e_pp.repeat(3), (e_tt * 3).repeat(3)
                          + np.tile([0, 1, 2], len(edges))] = fyo_np[src[edges]].ravel()
        per_core.append(dict(ea_s=ea_s, idx_s=np.tile(idx_s, (8, 1)),
                             dst_s=dst_s, na_s=na_s, fyo_s=fyo_s))

    # xc0 buffer: [N, 16] fp16 in x-row order: cols [x(5), sdf, na, 0...]
    x_np = np.asarray(inputs['x'], np.float32)
    sdf_np = np.asarray(inputs['sdf'], np.float32)
    xc0 = np.zeros((N, 128), np.float16)
    xc0[xrow, 0:x_np.shape[1]] = x_np
    xc0[xrow, x_np.shape[1]] = sdf_np[:, 0]
    xc0[xrow, x_np.shape[1] + 1] = na_np

    # weights per layer (fp16, reordered to kernel d-layout)
    wts = {}
    for lay in cfg.LAYERS:
        nm = lay['name']
        win = np.asarray(inputs[f'win_{nm}'], np.float32)
        bin_ = np.asarray(inputs[f'bin_{nm}'], np.float32)
        wout = np.asarray(inputs[f'wout_{nm}'], np.float32)
        bout = np.asarray(inputs[f'bout_{nm}'], np.float32)
        m = _col2orig(cfg, lay)
        D, OCP = lay['D'], lay['OCP']
        winT = np.zeros((7, D), np.float16)
        sel = m >= 0
        winT[0:cfg.EA, sel] = win[m[sel]].T
        winT[6, sel] = bin_[m[sel]]
        woutT = np.zeros((D + 1, OCP), np.float16)
        woutT[np.where(sel)[0], 0:lay['oc']] = wout[:, m[sel]].T
        woutT[D, 0:lay['oc']] = bout
        wts[f'winT_{nm}'] = winT
        wts[f'woutT_{nm}'] = woutT

    struct = dict(kA=kA, kB=kB, tw=tw, tbase=tbase, T=T,
                  TWMAX=int(tw.max()))
    asm = dict(node_core=node_core, node_row=node_row)
    return struct, per_core, wts, xc0, asm


def _build(cfg, struct):
    kA, kB, twin, tbase, T = (struct['kA'], struct['kB'], struct['tw'],
                              struct['tbase'], struct['T'])
    TWMAX = struct['TWMAX']
    HID = cfg.HID
    NW = cfg.NWIN
    DMAX = max(l['D'] for l in cfg.LAYERS)

    nc = bacc.Bacc("TRN2", target_bir_lowering=False, debug=False,
                   enable_asserts=True, num_devices=NCORES,
                   num_swdge_queues=4)
    ea_in = nc.dram_tensor("ea_s", [7, T * 128], F16, kind="ExternalInput").ap()
    idx_in = nc.dram_tensor("idx_s", [128, T * 8], I16, kind="ExternalInput").ap()
    dst_in = nc.dram_tensor("dst_s", [128, T], F16, kind="ExternalInput").ap()
    na_in = nc.dram_tensor("na_s", [128, T], F16, kind="ExternalInput").ap()
    fyo_in = nc.dram_tensor("fyo_s", [128, 3 * T], F16, kind="ExternalInput").ap()
    xc0_in = nc.dram_tensor("xc0_in", [cfg.N, 128], F16, kind="ExternalInput").ap()
    win_ins = {}
    wout_ins = {}
    for lay in cfg.LAYERS:
        nm = lay['name']
        win_ins[nm] = nc.dram_tensor(f"winT_{nm}", [7, lay['D']], F16,
                                     kind="ExternalInput").ap()
        wout_ins[nm] = nc.dram_tensor(f"woutT_{nm}", [lay['D'] + 1, lay['OCP']],
                                      F16, kind="ExternalInput").ap()
    out_fin = nc.dram_tensor("out_final", [cfg.NPC, cfg.OUT], F32,
                             kind="ExternalOutput").ap()

    with tile.TileContext(nc) as tc:
        with (
            tc.tile_pool(name="cst", bufs=1) as cst,
            tc.tile_pool(name="sbw", bufs=2) as sbw,
            tc.tile_pool(name="gst", bufs=4) as gst,
            tc.tile_pool(name="eap", bufs=4) as eap,
            tc.tile_pool(name="msgp", bufs=2) as msgp,
            tc.tile_pool(name="scp", bufs=3) as scp,
            tc.tile_pool(name="Sp", bufs=4) as Sp,
            tc.tile_pool(name="agsp", bufs=2) as agsp,
            tc.tile_pool(name="agtp", bufs=2) as agtp,
            tc.tile_pool(name="outp", bufs=2) as outp,
            tc.tile_pool(name="ps_sc", bufs=3, space="PSUM") as ps_sc,
            tc.tile_pool(name="ps_ag", bufs=1, space="PSUM") as ps_ag,
            tc.tile_pool(name="ps_tp", bufs=1, space="PSUM") as ps_tp,
            tc.tile_pool(name="dram", bufs=1, space="DRAM") as dram,
        ):
            # ---- constants
            iota_i = cst.tile([128, 128], I32)
            nc.gpsimd.iota(iota_i[:, :], pattern=[[1, 128]], base=0,
                           channel_multiplier=0)
            iota_f = cst.tile([128, 128], F16)
            nc.vector.tensor_copy(iota_f[:, :], iota_i[:, :])
            iota_p = cst.tile([128, 1], I32)
            nc.gpsimd.iota(iota_p[:, :], pattern=[[1, 1]], base=0,
                           channel_multiplier=1)
            iota_pf = cst.tile([128, 1], F32)
            nc.vector.tensor_copy(iota_pf[:, :], iota_p[:, :])
            identf = cst.tile([128, 128], F32)
            nc.vector.tensor_scalar(out=identf[:, :], in0=iota_f[:, :],
                                    scalar1=iota_pf[:, :], scalar2=None,
                                    op0=ISEQ)
            ident = cst.tile([128, 128], F16)
            nc.vector.tensor_copy(ident[:, :], identf[:, :])
            ones_i = cst.tile([1, 128], I32)
            nc.gpsimd.iota(ones_i[:, :], pattern=[[0, 128]], base=1,
                           channel_multiplier=0)
            ones = cst.tile([1, 128], F16)
            nc.vector.tensor_copy(ones[:, :], ones_i[:, :])
            zeros = cst.tile([128, 512], F16)
            nc.vector.memset(zeros[:, :], 0)

            # ---- static per-slot data (resident)
            dst_sb = cst.tile([128, T], F16)
            nc.sync.dma_start(out=dst_sb[:, :], in_=dst_in[:, :])
            na_sb = cst.tile([128, T], F16)
            nc.sync.dma_start(out=na_sb[:, :], in_=na_in[:, :])
            fyo_sb = cst.tile([128, 3 * T], F16)
            nc.sync.dma_start(out=fyo_sb[:, :], in_=fyo_in[:, :])
            idx_sb = cst.tile([128, T * 8], I16)
            nc.sync.dma_start(out=idx_sb[:, :], in_=idx_in[:, :])

            # ---- DRAM buffers
            xc0b = dram.tile([cfg.N, 128], F16)
            nc.sync.dma_start(out=xc0b[:, :], in_=xc0_in[:, :])
            xstate = {}
            cbase = np.concatenate(
                [[0], np.cumsum([r * NCORES for r in cfg.CROWS])]).astype(int)

            def emit_layer(lay):
                nm, D, OCP, TWL = lay['name'], lay['D'], lay['OCP'], lay['tw']
                dmain = lay['dmain']
                gcols = lay['gcols']
                nfull = dmain // 128                     # 12 or 0
                # out-mm K chunks: nfull x 128 + tail TWL
                if nm == 'p0':
                    gsrc = xc0b
                else:
                    gsrc = xstate['cur']

                # layer weights (fp16 direct)
                winT = sbw.tile([7, DMAX], F16, tag="winT")
                nc.sync.dma_start(out=winT[:, 0:D], in_=win_ins[nm][:, :])
                wt = sbw.tile([128, 12 * 512], F16, tag="wt")
                for ci in range(nfull):
                    nc.sync.dma_start(
                        out=wt[:, ci * OCP:ci * OCP + OCP],
                        in_=wout_ins[nm][ci * 128:(ci + 1) * 128, :])
                wtail = sbw.tile([32, 512], F16, tag="wtail")
                nc.sync.dma_start(
                    out=wtail[0:TWL, 0:OCP],
                    in_=wout_ins[nm][nfull * 128:nfull * 128 + TWL, :])
                wbias = sbw.tile([1, 512], F16, tag="wbias")
                nc.sync.dma_start(out=wbias[:, 0:OCP],
                                  in_=wout_ins[nm][D:D + 1, :])

                if nm != 'c2':
                    Xout = dram.tile([cfg.N, HID], F16,
                                     tag="Xbuf", name=f"X_{nm}", bufs=2)
                    oslices = []
                    for k, r in enumerate(cfg.CROWS):
                        t_ = dram.tile([r, HID], F16, tag=f"osl_{k}",
                                       name=f"osl_{nm}_{k}", bufs=1)
                        oslices.append(t_)
                    xstate['cur'] = Xout

                # pipeline state per window
                state = {}

                def st_gather(w):
                    nt = int(twin[w])
                    t0 = int(tbase[w])
                    ka, kb = int(kA[w]), int(kB[w])
                    xst = gst.tile([128, TWMAX, gcols], F16, tag=f"xst{gcols}")
                    if ka:
                        nc.gpsimd.dma_gather(
                            out_ap=xst[:, 0:ka, 0:gcols],
                            in_ap=gsrc[0:cfg.VIEW, :],
                            idxs_ap=idx_sb[:, t0 * 8:(t0 + ka) * 8],
                            num_idxs=ka * 128, num_idxs_reg=ka * 128,
                            elem_size=gcols,
                            queue_num=(2 * w) % 4)
                    if kb:
                        nc.gpsimd.dma_gather(
                            out_ap=xst[:, ka:nt, 0:gcols],
                            in_ap=gsrc[cfg.ABOFS:cfg.ABOFS + cfg.VIEW, :],
                            idxs_ap=idx_sb[:, (t0 + ka) * 8:(t0 + nt) * 8],
                            num_idxs=kb * 128, num_idxs_reg=kb * 128,
                            elem_size=gcols,
                            queue_num=(2 * w + 1) % 4)
                    eaf = eap.tile([7, TWMAX * 128], F16, tag="eaf")
                    nc.sync.dma_start(out=eaf[:, 0:nt * 128],
                                      in_=ea_in[:, t0 * 128:(t0 + nt) * 128])
                    S = Sp.tile([128, TWMAX * 128], F16, tag="S")
                    nc.vector.tensor_tensor(
                        out=S[:, 0:nt * 128].rearrange("p (t n) -> p t n", n=128),
                        in0=iota_f[:, :].unsqueeze(1).broadcast_to([128, nt, 128]),
                        in1=dst_sb[:, t0:t0 + nt].unsqueeze(2).broadcast_to(
                            [128, nt, 128]),
                        op=ISEQ)
                    state[w] = dict(xst=xst, eaf=eaf, S=S)

                def st_scaling_msg(w, scat_w):
                    """Interleave scaling MMs + fused relu-mult for window w
                    with scatter MMs for window scat_w (PE never waits)."""
                    nt = int(twin[w]) if w is not None else 0
                    t0 = int(tbase[w]) if w is not None else 0
                    if w is not None:
                        sw = state[w]
                        msg = msgp.tile([128, TWMAX, DMAX], F16, tag="msg")
                        sw['msg'] = msg
                    # scatter stream for scat_w
                    scat_ops = []
                    if scat_w is not None:
                        ssw = state[scat_w]
                        snt = int(twin[scat_w])
                        agps = ps_ag.tile([128, 2048], F32, tag="agps")
                        ssw['agps'] = agps
                        smsg = ssw['msg']
                        sS = ssw['S']
                        npieces = (3 if dmain else 0) + 1
                        for t in range(snt):
                            for pi in range(npieces):
                                if dmain and pi < 3:
                                    lo, hi = pi * 512, (pi + 1) * 512
                                else:
                                    lo, hi = dmain, dmain + TWL
                                scat_ops.append((t, lo, hi))
                        scat_i = [0]

                        def emit_scat(n=1):
                            for _ in range(n):
                                if scat_i[0] >= len(scat_ops):
                                    return
                                t, lo, hi = scat_ops[scat_i[0]]
                                scat_i[0] += 1
                                nc.tensor.matmul(
                                    agps[:, lo:hi],
                                    sS[:, t * 128:(t + 1) * 128],
                                    smsg[:, t, lo:hi],
                                    start=(t == 0), stop=(t == snt - 1))
                    else:
                        def emit_scat(n=1):
                            return

                    # scaling + fused relu*x for w, interleaved
                    eng_i = [0]
                    for t in range(nt):
                        tg = t0 + t
                        pieces = ([(p * 512, (p + 1) * 512) for p in range(3)]
                                  if dmain else [])
                        pieces.append((dmain, dmain + TWL))
                        for (lo, hi) in pieces:
                            scps = ps_sc.tile([128, 512], F32, tag="scps")
                            nc.tensor.matmul(
                                scps[:, 0:hi - lo],
                                sw['eaf'][:, t * 128:(t + 1) * 128],
                                winT[:, lo:hi], start=True, stop=True)
                            emit_scat(1)
                            # msg = relu(scps) * x_j  (GPSIMD can't touch
                            # PSUM: path A = DVE fused STT; B/C = ACT relu
                            # then DVE-2x / GPSIMD multiply)
                            eng = nc.vector
                            if dmain and hi <= dmain:
                                path = 'ACB'[eng_i[0] % 3]
                                eng_i[0] += 1
                                if path == 'A':
                                    nc.vector.scalar_tensor_tensor(
                                        out=msg[:, t, lo:hi],
                                        in0=scps[:, 0:512],
                                        scalar=0.0, in1=sw['xst'][:, t, 0:512],
                                        op0=MAX, op1=MULT)
                                else:
                                    sc = scp.tile([128, 512], F16, tag="sc")
                                    nc.scalar.activation(sc[:, :],
                                                         scps[:, 0:512], RELU)
                                    meng = nc.vector if path == 'B' else nc.gpsimd
                                    meng.tensor_tensor(
                                        out=msg[:, t, lo:hi], in0=sc[:, :],
                                        in1=sw['xst'][:, t, 0:512], op=MULT)
                            elif nm == 'p0':
                                eng.scalar_tensor_tensor(
                                    out=msg[:, t, 0:24].rearrange(
                                        "p (h i) -> p h i", i=8),
                                    in0=scps[:, 0:24].rearrange(
                                        "p (h i) -> p h i", i=8),
                                    scalar=0.0,
                                    in1=sw['xst'][:, t, 0:8].unsqueeze(1)
                                    .broadcast_to([128, 3, 8]),
                                    op0=MAX, op1=MULT)
                            else:
                                # tail: na (3) [+ fyo (9) before na for c0]
                                if nm == 'c0':
                                    eng.scalar_tensor_tensor(
                                        out=msg[:, t, dmain:dmain + 9].rearrange(
                                            "p (f h) -> p f h", h=3),
                                        in0=scps[:, 0:9].rearrange(
                                            "p (f h) -> p f h", h=3),
                                        scalar=0.0,
                                        in1=fyo_sb[:, 3 * tg:3 * tg + 3]
                                        .unsqueeze(2).broadcast_to([128, 3, 3]),
                                        op0=MAX, op1=MULT)
                                    nlo, ntw = 9, 3
                                else:
                                    nlo, ntw = 0, TWL
                                eng.scalar_tensor_tensor(
                                    out=msg[:, t, dmain + nlo:dmain + nlo + ntw],
                                    in0=scps[:, nlo:nlo + ntw],
                                    scalar=0.0,
                                    in1=na_sb[:, tg:tg + 1].broadcast_to(
                                        [128, ntw]),
                                    op0=MAX, op1=MULT)
                    emit_scat(10000)

                def st_evac(w):
                    ssw = state[w]
                    ags = agsp.tile([128, DMAX], F16, tag="ags")
                    nc.scalar.activation(ags[:, 0:D], ssw['agps'][:, 0:D], COPY)
                    ssw['ags'] = ags

                def st_transpose(w):
                    ssw = state[w]
                    ags = ssw['ags']
                    agT = agtp.tile([128, 12 * 128], F16, tag="agT")
                    agTt = agtp.tile([32, 128], F16, tag="agTt")
                    for gi in range(0, nfull, 8):
                        gn = min(8, nfull - gi)
                        tp = ps_tp.tile([128, 1024], F16, tag="tp")
                        for u in range(gn):
                            ci = gi + u
                            nc.tensor.transpose(
                                tp[:, u * 128:(u + 1) * 128],
                                ags[:, ci * 128:(ci + 1) * 128],
                                ident[:, :])
                        eng = nc.vector if gi == 0 else nc.scalar
                        if eng is nc.vector:
                            nc.vector.tensor_copy(
                                agT[:, gi * 128:(gi + gn) * 128],
                                tp[:, 0:gn * 128])
                        else:
                            nc.scalar.activation(
                                agT[:, gi * 128:(gi + gn) * 128],
                                tp[:, 0:gn * 128], COPY)
                    tp2 = ps_tp.tile([128, 1024], F16, tag="tp")
                    nc.tensor.transpose(
                        tp2[0:TWL, 0:128],
                        ags[:, dmain:dmain + TWL],
                        ident[:, :])
                    nc.vector.tensor_copy(agTt[0:TWL, :], tp2[0:TWL, 0:128])
                    ssw['agT'] = agT
                    ssw['agTt'] = agTt

                def st_outmm(w, wa, k):
                    ssw = state[w]
                    wsz = cfg.WSIZES[w]
                    omf = ps_tp.tile([128, 512], F32, tag="tp")
                    for ci in range(nfull):
                        nc.tensor.matmul(
                            omf[:, 0:OCP],
                            ssw['agT'][:, ci * 128:(ci + 1) * 128],
                            wt[:, ci * OCP:(ci + 1) * OCP],
                            start=(ci == 0), stop=False)
                    nc.tensor.matmul(omf[:, 0:OCP], ssw['agTt'][0:TWL, :],
                                     wtail[0:TWL, 0:OCP],
                                     start=(nfull == 0), stop=False)
                    nc.tensor.matmul(omf[:, 0:OCP], ones[:, :],
                                     wbias[:, 0:OCP], start=False, stop=True)
                    odt = F32 if nm == 'c2' else F16
                    outs = outp.tile([128, 512], odt, tag="outs")
                    nc.scalar.activation(outs[:, 0:OCP], omf[:, 0:OCP], TANH)
                    if nm == 'c2':
                        nc.sync.dma_start(
                            out=out_fin[w * 128:w * 128 + wsz, :],
                            in_=outs[0:wsz, 0:cfg.OUT])
                    else:
                        outr = outp.tile([128, 512], F16, tag="outr")
                        nc.vector.tensor_tensor(out=outr[:, 0:OCP],
                                                in0=outs[:, 0:OCP],
                                                in1=zeros[:, 0:OCP], op=MAX)
                        ro = sum(cfg.WSIZES[wa:w])
                        nc.sync.dma_start(
                            out=oslices[k][ro:ro + wsz, :],
                            in_=outr[0:wsz, 0:HID])
                    del state[w]

                # window -> chunk lookup
                w_chunk = {}
                for k, (wa, wb) in enumerate(cfg.WCHUNKS):
                    for w in range(wa, wb):
                        w_chunk[w] = (k, wa, wb)

                for it in range(NW + 4):
                    if 0 <= it - 3 < NW:
                        st_transpose(it - 3)
                    if 0 <= it - 4 < NW:
                        w4 = it - 4
                        k, wa, wb = w_chunk[w4]
                        st_outmm(w4, wa, k)
                        # AllGather when the chunk's last window retires
                        if nm != 'c2' and w4 == wb - 1:
                            r = cfg.CROWS[k]
                            nc.gpsimd.collective_compute(
                                "AllGather", mybir.AluOpType.bypass,
                                replica_groups=[list(range(NCORES))],
                                ins=[oslices[k][:, :]],
                                outs=[Xout[cbase[k]:cbase[k] + NCORES * r, :]])
                    w1 = it - 1 if 0 <= it - 1 < NW else None
                    w2 = it - 2 if 0 <= it - 2 < NW else None
                    if w1 is not None or w2 is not None:
                        st_scaling_msg(w1, w2)
                    if w2 is not None:
                        st_evac(w2)
                    if it == 0:
                        st_gather(0)
                    if it + 1 < NW:
                        st_gather(it + 1)

            for lay in cfg.LAYERS:
                emit_layer(lay)
    nc.compile()
    return nc


def _run(inputs, trace=False):
    cfg = Cfg()
    struct, per_core, wts, xc0, asm = _preprocess(cfg, inputs)
    nc = _build(cfg, struct)
    in_maps = []
    for c in range(NCORES):
        im = dict(per_core[c])
        im['xc0_in'] = xc0
        for k, v in wts.items():
            im[k] = v
        in_maps.append(im)
    res = run_bass_kernel_spmd(nc, in_maps, list(range(NCORES)), trace=trace)
    out = np.zeros((cfg.N, cfg.OUT), np.float32)
    for c in range(NCORES):
        sl = res.results[c]['out_final']
        sel = asm['node_core'] == c
        out[sel] = sl[asm['node_row'][sel]]
    return out, res


def kernel(**inputs):
    return _run(inputs, trace=False)[0]


# revision 18
# speedup vs baseline: 1.8292x; 1.4594x over previous
"""Trainium2 Bass kernel for nn_CFDFVnewGCN (6-layer FVnewConv GNN).

Strategy: shard destination nodes (and their incoming edges) across 8 cores.
Nodes are permuted/degree-balanced into 49 windows of <=128 nodes per core.
fp16 datapath end-to-end (PSUM accumulation stays f32).  Per 128-edge tile:
scaling matmul (edge_attr stationary, bias folded as 8th K-row) into PSUM,
fused relu*x_j via scalar_tensor_tensor (PSUM -> fp16 msg, split between DVE
and GPSIMD), one-hot scatter matmul accumulating aggr[window, D] in PSUM.
Per window: PE transposes of aggr + output matmul (bias via const ones row),
tanh (ACT) + relu (DVE).  d-layout: 3 planes of 512 gathered x cols
(d = h*512 + i), then a small tail (na, and fyo for c0) so the scatter PSUM
is exactly 4 banks.  Software pipeline: at iteration `it` we emit
gather(it) / scaling+msg(it-1) interleaved with scatter(it-2) /
transposes(it-3) / out-matmul(it-4), keeping the PE stream dependency-free
and continuously busy (full 2.4 GHz p-state).  x replicated via 4 chunked
AllGathers per layer (overlapping compute) into a Shared DRAM buffer.
Gather uses dma_gather with two offset views of the x buffer (rows 0:32768
and 17232:50000) so indices fit int16.
"""
import sys
import numpy as np

for _p in ('/opt/trn_rl_repo', '/root/.axon_site/_ro/trn_rl_repo'):
    if _p not in sys.path:
        sys.path.insert(0, _p)

import concourse.bacc as bacc
import concourse.bass as bass
import concourse.mybir as mybir
import concourse.tile as tile
from concourse.bass_utils import run_bass_kernel_spmd

F32 = mybir.dt.float32
F16 = mybir.dt.float16
I16 = mybir.dt.int16
I32 = mybir.dt.int32
COPY = mybir.ActivationFunctionType.Copy
RELU = mybir.ActivationFunctionType.Relu
TANH = mybir.ActivationFunctionType.Tanh
MULT = mybir.AluOpType.mult
MAX = mybir.AluOpType.max
ISEQ = mybir.AluOpType.is_equal

NCORES = 8
G = 512          # gathered x columns (one plane width)
DMAIN = 3 * G    # 1536


class Cfg:
    def __init__(self, n_nodes=50000, n_edges=200000, hid=512, hs=3, ea=6,
                 out=3, n_ag_chunks=4):
        self.N = n_nodes
        self.E = n_edges
        self.HID = hid
        self.HS = hs
        self.EA = ea
        self.OUT = out
        self.NPC = self.N // NCORES              # nodes per core
        self.NWIN = (self.NPC + 127) // 128      # windows per core
        self.WSIZES = [128] * (self.NWIN - 1) + [self.NPC - 128 * (self.NWIN - 1)]
        # A/B view split of the x buffer rows (int16 gather index range)
        self.VIEW = min(32768, self.N)
        self.ABOFS = max(0, self.N - self.VIEW)
        # AllGather chunking: split windows into n_ag_chunks groups
        if n_ag_chunks == 4 and self.NWIN == 49:
            bounds = [0, 20, 35, 45, 49]
        else:
            k = min(n_ag_chunks, self.NWIN)
            bounds = [round(i * self.NWIN / k) for i in range(k + 1)]
        self.WCHUNKS = [(bounds[i], bounds[i + 1]) for i in range(len(bounds) - 1)]
        self.CROWS = [sum(self.WSIZES[a:b]) for a, b in self.WCHUNKS]
        # layer table
        #  p0: ic=7  ([x5,sdf,na] + pad), gathered from xc0 (16 fp16 cols)
        #  others: 3 planes of 512 x-cols + tail (na=3; c0 adds fyo=9)
        self.LAYERS = []
        for name in ['p0', 'p1', 'p2', 'c0', 'c1', 'c2']:
            if name == 'p0':
                lay = dict(name=name, oc=hid, dmain=0, tw=24, gcols=128,
                           relu=True)
            else:
                oc = out if name == 'c2' else hid
                tw = 12 if name == 'c0' else 4
                lay = dict(name=name, oc=oc, dmain=DMAIN, tw=tw, gcols=G,
                           relu=(name != 'c2'))
            lay['D'] = lay['dmain'] + lay['tw']
            lay['OCP'] = lay['oc'] + (-lay['oc']) % 4
            self.LAYERS.append(lay)


def _col2orig(cfg, lay):
    """Map new d-layout column -> original scaling index j=i*HS+h, -1=pad."""
    HS = cfg.HS
    m = np.full(lay['D'], -1, np.int64)
    nm = lay['name']
    if nm == 'p0':
        # d = h*8 + i, i<7 ([x5, sdf, na])
        for h in range(HS):
            for i in range(7):
                m[h * 8 + i] = i * HS + h
    else:
        # ic layout in reference xc: c0: [fyo3, x512, na1]; else [x512, na1]
        xoff = 3 if nm == 'c0' else 0
        for h in range(HS):
            for i in range(G):
                m[h * G + i] = (xoff + i) * HS + h
        t0 = DMAIN
        if nm == 'c0':
            for f in range(3):
                for h in range(HS):
                    m[t0 + 3 * f + h] = f * HS + h        # fyo
            for h in range(HS):
                m[t0 + 9 + h] = (3 + G) * HS + h          # na
        else:
            for h in range(HS):
                m[t0 + h] = G * HS + h                    # na
    return m


def _balance(items_deg, caps):
    """Greedy: assign items (sorted by degree desc) to bins with capacity,
    minimizing max degree sum. Returns bin index per item."""
    order = np.argsort(-items_deg, kind='stable')
    nbins = len(caps)
    load = np.zeros(nbins)
    cnt = np.zeros(nbins, np.int64)
    out = np.zeros(len(items_deg), np.int64)
    import heapq
    heap = [(0.0, b) for b in range(nbins)]
    heapq.heapify(heap)
    for it in order:
        while True:
            l, b = heapq.heappop(heap)
            if cnt[b] < caps[b]:
                break
        out[it] = b
        cnt[b] += 1
        load[b] += items_deg[it]
        if cnt[b] < caps[b]:
            heapq.heappush(heap, (load[b], b))
    return out


def _preprocess(cfg, inputs):
    N, E, HS = cfg.N, cfg.E, cfg.HS
    ei = np.asarray(inputs['edge_index'])
    src = ei[0].astype(np.int64)
    dst = ei[1].astype(np.int64)
    deg = np.bincount(dst, minlength=N).astype(np.float64)

    node_core = _balance(deg, [cfg.NPC] * NCORES)
    node_win = np.zeros(N, np.int64)
    node_slot = np.zeros(N, np.int64)
    for c in range(NCORES):
        nodes = np.where(node_core == c)[0]
        w = _balance(deg[nodes], cfg.WSIZES)
        node_win[nodes] = w
        for wi in range(cfg.NWIN):
            sel = nodes[w == wi]
            node_slot[sel] = np.arange(len(sel))

    # within-core row and global x row (AG chunk-major, rank-interleaved)
    node_row = node_win * 128 + node_slot
    cbase = np.concatenate([[0], np.cumsum([r * NCORES for r in cfg.CROWS])])
    wchunk = np.zeros(cfg.NWIN, np.int64)
    wofs = np.zeros(cfg.NWIN, np.int64)
    for k, (a, b) in enumerate(cfg.WCHUNKS):
        for w in range(a, b):
            wchunk[w] = k
            wofs[w] = sum(cfg.WSIZES[a:w])
    k_of = wchunk[node_win]
    xrow = (cbase[k_of] + node_core * np.array(cfg.CROWS)[k_of]
            + wofs[node_win] + node_slot)
    xrow_src = xrow[src]

    # edge buckets per (core, window)
    ec = node_core[dst]
    ew = node_win[dst]

    # per-window global tile structure (max over cores)
    tw = np.zeros(cfg.NWIN, np.int64)
    cntT = np.zeros((NCORES, cfg.NWIN), np.int64)
    np.add.at(cntT, (ec, ew), 1)
    for w in range(cfg.NWIN):
        tw[w] = max(int(np.ceil(cntT[:, w].max() / 128)), 1)
    tbase = np.concatenate([[0], np.cumsum(tw)])
    T = int(tbase[-1])

    ea_np = np.asarray(inputs['edge_attr'], np.float32)
    na_np = np.asarray(inputs['node_attr'], np.float32).reshape(-1)
    fyo_np = np.asarray(inputs['fine_y_orig'], np.float32)

    per_core = []
    for c in range(NCORES):
        ea_s = np.zeros((7, T * 128), np.float16)
        idx_s = np.zeros((128, T), np.int32)
        dst_s = np.full((128, T), 999.0, np.float16)
        na_s = np.zeros((128, T), np.float16)
        fyo_s = np.zeros((128, 3 * T), np.float16)
        for w in range(cfg.NWIN):
            edges = np.where((ec == c) & (ew == w))[0]
            t0 = int(tbase[w])
            assert len(edges) <= tw[w] * 128
            jj = np.arange(len(edges))
            e_tt = t0 + jj // 128
            e_pp = jj % 128
            idx_s[e_pp, e_tt] = xrow_src[edges].astype(np.int32)
            ea_s[0:6, e_tt * 128 + e_pp] = ea_np[edges].T
            ea_s[6, e_tt * 128 + e_pp] = 1.0
            dst_s[e_pp, e_tt] = node_slot[dst[edges]]
            na_s[e_pp, e_tt] = na_np[src[edges]]
            fyo_s[e_pp.repeat(3), (e_tt * 3).repeat(3)
                  + np.tile([0, 1, 2], len(edges))] = fyo_np[src[edges]].ravel()
        per_core.append(dict(ea_s=ea_s, idx_s=idx_s,
                             dst_s=dst_s, na_s=na_s, fyo_s=fyo_s))

    # xc0 buffer: [N, 16] fp16 in x-row order: cols [x(5), sdf, na, 0...]
    x_np = np.asarray(inputs['x'], np.float32)
    sdf_np = np.asarray(inputs['sdf'], np.float32)
    xc0 = np.zeros((N, 128), np.float16)
    xc0[xrow, 0:x_np.shape[1]] = x_np
    xc0[xrow, x_np.shape[1]] = sdf_np[:, 0]
    xc0[xrow, x_np.shape[1] + 1] = na_np

    # weights per layer (fp16, reordered to kernel d-layout)
    wts = {}
    for lay in cfg.LAYERS:
        nm = lay['name']
        win = np.asarray(inputs[f'win_{nm}'], np.float32)
        bin_ = np.asarray(inputs[f'bin_{nm}'], np.float32)
        wout = np.asarray(inputs[f'wout_{nm}'], np.float32)
        bout = np.asarray(inputs[f'bout_{nm}'], np.float32)
        m = _col2orig(cfg, lay)
        D, OCP = lay['D'], lay['OCP']
        winT = np.zeros((7, D), np.float16)
        sel = m >= 0
        winT[0:cfg.EA, sel] = win[m[sel]].T
        winT[6, sel] = bin_[m[sel]]
        woutT = np.zeros((D + 1, OCP), np.float16)
        woutT[np.where(sel)[0], 0:lay['oc']] = wout[:, m[sel]].T
        woutT[D, 0:lay['oc']] = bout
        wts[f'winT_{nm}'] = winT
        wts[f'woutT_{nm}'] = woutT

    struct = dict(tw=tw, tbase=tbase, T=T, TWMAX=int(tw.max()))
    asm = dict(node_core=node_core, node_row=node_row)
    return struct, per_core, wts, xc0, asm


def _build(cfg, struct):
    twin, tbase, T = struct['tw'], struct['tbase'], struct['T']
    TWMAX = struct['TWMAX']
    HID = cfg.HID
    NW = cfg.NWIN
    DMAX = max(l['D'] for l in cfg.LAYERS)

    nc = bacc.Bacc("TRN2", target_bir_lowering=False, debug=False,
                   enable_asserts=True, num_devices=NCORES,
                   num_swdge_queues=4)
    ea_in = nc.dram_tensor("ea_s", [7, T * 128], F16, kind="ExternalInput").ap()
    idx_in = nc.dram_tensor("idx_s", [128, T], I32, kind="ExternalInput").ap()
    dst_in = nc.dram_tensor("dst_s", [128, T], F16, kind="ExternalInput").ap()
    na_in = nc.dram_tensor("na_s", [128, T], F16, kind="ExternalInput").ap()
    fyo_in = nc.dram_tensor("fyo_s", [128, 3 * T], F16, kind="ExternalInput").ap()
    xc0_in = nc.dram_tensor("xc0_in", [cfg.N, 128], F16, kind="ExternalInput").ap()
    win_ins = {}
    wout_ins = {}
    for lay in cfg.LAYERS:
        nm = lay['name']
        win_ins[nm] = nc.dram_tensor(f"winT_{nm}", [7, lay['D']], F16,
                                     kind="ExternalInput").ap()
        wout_ins[nm] = nc.dram_tensor(f"woutT_{nm}", [lay['D'] + 1, lay['OCP']],
                                      F16, kind="ExternalInput").ap()
    out_fin = nc.dram_tensor("out_final", [cfg.NPC, cfg.OUT], F32,
                             kind="ExternalOutput").ap()

    with tile.TileContext(nc) as tc:
        with (
            tc.tile_pool(name="cst", bufs=1) as cst,
            tc.tile_pool(name="sbw", bufs=2) as sbw,
            tc.tile_pool(name="gst", bufs=4) as gst,
            tc.tile_pool(name="eap", bufs=4) as eap,
            tc.tile_pool(name="msgp", bufs=2) as msgp,
            tc.tile_pool(name="scp", bufs=3) as scp,
            tc.tile_pool(name="Sp", bufs=4) as Sp,
            tc.tile_pool(name="agsp", bufs=2) as agsp,
            tc.tile_pool(name="agtp", bufs=2) as agtp,
            tc.tile_pool(name="outp", bufs=2) as outp,
            tc.tile_pool(name="ps_sc", bufs=3, space="PSUM") as ps_sc,
            tc.tile_pool(name="ps_ag", bufs=1, space="PSUM") as ps_ag,
            tc.tile_pool(name="ps_tp", bufs=1, space="PSUM") as ps_tp,
            tc.tile_pool(name="dram", bufs=1, space="DRAM") as dram,
        ):
            # ---- constants
            iota_i = cst.tile([128, 128], I32)
            nc.gpsimd.iota(iota_i[:, :], pattern=[[1, 128]], base=0,
                           channel_multiplier=0)
            iota_f = cst.tile([128, 128], F16)
            nc.vector.tensor_copy(iota_f[:, :], iota_i[:, :])
            iota_p = cst.tile([128, 1], I32)
            nc.gpsimd.iota(iota_p[:, :], pattern=[[1, 1]], base=0,
                           channel_multiplier=1)
            iota_pf = cst.tile([128, 1], F32)
            nc.vector.tensor_copy(iota_pf[:, :], iota_p[:, :])
            identf = cst.tile([128, 128], F32)
            nc.vector.tensor_scalar(out=identf[:, :], in0=iota_f[:, :],
                                    scalar1=iota_pf[:, :], scalar2=None,
                                    op0=ISEQ)
            ident = cst.tile([128, 128], F16)
            nc.vector.tensor_copy(ident[:, :], identf[:, :])
            ones_i = cst.tile([1, 128], I32)
            nc.gpsimd.iota(ones_i[:, :], pattern=[[0, 128]], base=1,
                           channel_multiplier=0)
            ones = cst.tile([1, 128], F16)
            nc.vector.tensor_copy(ones[:, :], ones_i[:, :])
            zeros = cst.tile([128, 512], F16)
            nc.vector.memset(zeros[:, :], 0)

            # ---- static per-slot data (resident)
            dst_sb = cst.tile([128, T], F16)
            nc.sync.dma_start(out=dst_sb[:, :], in_=dst_in[:, :])
            na_sb = cst.tile([128, T], F16)
            nc.sync.dma_start(out=na_sb[:, :], in_=na_in[:, :])
            fyo_sb = cst.tile([128, 3 * T], F16)
            nc.sync.dma_start(out=fyo_sb[:, :], in_=fyo_in[:, :])
            idx_sb = cst.tile([128, T], I32)
            nc.sync.dma_start(out=idx_sb[:, :], in_=idx_in[:, :])

            # ---- DRAM buffers
            xc0b = dram.tile([cfg.N, 128], F16)
            nc.sync.dma_start(out=xc0b[:, :], in_=xc0_in[:, :])
            xstate = {}
            cbase = np.concatenate(
                [[0], np.cumsum([r * NCORES for r in cfg.CROWS])]).astype(int)

            def emit_layer(lay):
                nm, D, OCP, TWL = lay['name'], lay['D'], lay['OCP'], lay['tw']
                dmain = lay['dmain']
                gcols = lay['gcols']
                nfull = dmain // 128                     # 12 or 0
                # out-mm K chunks: nfull x 128 + tail TWL
                if nm == 'p0':
                    gsrc = xc0b
                else:
                    gsrc = xstate['cur']

                # layer weights (fp16 direct)
                winT = sbw.tile([7, DMAX], F16, tag="winT")
                nc.sync.dma_start(out=winT[:, 0:D], in_=win_ins[nm][:, :])
                wt = sbw.tile([128, 12 * 512], F16, tag="wt")
                for ci in range(nfull):
                    nc.sync.dma_start(
                        out=wt[:, ci * OCP:ci * OCP + OCP],
                        in_=wout_ins[nm][ci * 128:(ci + 1) * 128, :])
                wtail = sbw.tile([32, 512], F16, tag="wtail")
                nc.sync.dma_start(
                    out=wtail[0:TWL, 0:OCP],
                    in_=wout_ins[nm][nfull * 128:nfull * 128 + TWL, :])
                wbias = sbw.tile([1, 512], F16, tag="wbias")
                nc.sync.dma_start(out=wbias[:, 0:OCP],
                                  in_=wout_ins[nm][D:D + 1, :])

                if nm != 'c2':
                    Xout = dram.tile([cfg.N, HID], F16,
                                     tag="Xbuf", name=f"X_{nm}", bufs=2)
                    oslices = []
                    for k, r in enumerate(cfg.CROWS):
                        t_ = dram.tile([r, HID], F16, tag=f"osl_{k}",
                                       name=f"osl_{nm}_{k}", bufs=1)
                        oslices.append(t_)
                    xstate['cur'] = Xout

                # pipeline state per window
                state = {}

                def st_gather(w):
                    nt = int(twin[w])
                    t0 = int(tbase[w])
                    xst = gst.tile([128, TWMAX, gcols], F16, tag=f"xst{gcols}")
                    for t in range(nt):
                        nc.gpsimd.indirect_dma_start(
                            out=xst[:, t, 0:gcols],
                            out_offset=None,
                            in_=gsrc[:, :],
                            in_offset=bass.IndirectOffsetOnAxis(
                                ap=idx_sb[:, t0 + t:t0 + t + 1], axis=0))
                    eaf = eap.tile([7, TWMAX * 128], F16, tag="eaf")
                    nc.sync.dma_start(out=eaf[:, 0:nt * 128],
                                      in_=ea_in[:, t0 * 128:(t0 + nt) * 128])
                    S = Sp.tile([128, TWMAX * 128], F16, tag="S")
                    nc.vector.tensor_tensor(
                        out=S[:, 0:nt * 128].rearrange("p (t n) -> p t n", n=128),
                        in0=iota_f[:, :].unsqueeze(1).broadcast_to([128, nt, 128]),
                        in1=dst_sb[:, t0:t0 + nt].unsqueeze(2).broadcast_to(
                            [128, nt, 128]),
                        op=ISEQ)
                    state[w] = dict(xst=xst, eaf=eaf, S=S)

                def st_scaling_msg(w, scat_w):
                    """Interleave scaling MMs + fused relu-mult for window w
                    with scatter MMs for window scat_w (PE never waits)."""
                    nt = int(twin[w]) if w is not None else 0
                    t0 = int(tbase[w]) if w is not None else 0
                    if w is not None:
                        sw = state[w]
                        msg = msgp.tile([128, TWMAX, DMAX], F16, tag="msg")
                        sw['msg'] = msg
                    # scatter stream for scat_w
                    scat_ops = []
                    if scat_w is not None:
                        ssw = state[scat_w]
                        snt = int(twin[scat_w])
                        agps = ps_ag.tile([128, 2048], F32, tag="agps")
                        ssw['agps'] = agps
                        smsg = ssw['msg']
                        sS = ssw['S']
                        npieces = (3 if dmain else 0) + 1
                        for t in range(snt):
                            for pi in range(npieces):
                                if dmain and pi < 3:
                                    lo, hi = pi * 512, (pi + 1) * 512
                                else:
                                    lo, hi = dmain, dmain + TWL
                                scat_ops.append((t, lo, hi))
                        scat_i = [0]

                        def emit_scat(n=1):
                            for _ in range(n):
                                if scat_i[0] >= len(scat_ops):
                                    return
                                t, lo, hi = scat_ops[scat_i[0]]
                                scat_i[0] += 1
                                nc.tensor.matmul(
                                    agps[:, lo:hi],
                                    sS[:, t * 128:(t + 1) * 128],
                                    smsg[:, t, lo:hi],
                                    start=(t == 0), stop=(t == snt - 1))
                    else:
                        def emit_scat(n=1):
                            return

                    # scaling + fused relu*x for w, interleaved
                    eng_i = [0]
                    for t in range(nt):
                        tg = t0 + t
                        pieces = ([(p * 512, (p + 1) * 512) for p in range(3)]
                                  if dmain else [])
                        pieces.append((dmain, dmain + TWL))
                        for (lo, hi) in pieces:
                            scps = ps_sc.tile([128, 512], F32, tag="scps")
                            nc.tensor.matmul(
                                scps[:, 0:hi - lo],
                                sw['eaf'][:, t * 128:(t + 1) * 128],
                                winT[:, lo:hi], start=True, stop=True)
                            emit_scat(1)
                            # msg = relu(scps) * x_j  (GPSIMD can't touch
                            # PSUM: path A = DVE fused STT; B/C = ACT relu
                            # then DVE-2x / GPSIMD multiply)
                            eng = nc.vector
                            if dmain and hi <= dmain:
                                path = 'ACB'[eng_i[0] % 3]
                                eng_i[0] += 1
                                if path == 'A':
                                    nc.vector.scalar_tensor_tensor(
                                        out=msg[:, t, lo:hi],
                                        in0=scps[:, 0:512],
                                        scalar=0.0, in1=sw['xst'][:, t, 0:512],
                                        op0=MAX, op1=MULT)
                                else:
                                    sc = scp.tile([128, 512], F16, tag="sc")
                                    nc.scalar.activation(sc[:, :],
                                                         scps[:, 0:512], RELU)
                                    meng = nc.vector if path == 'B' else nc.gpsimd
                                    meng.tensor_tensor(
                                        out=msg[:, t, lo:hi], in0=sc[:, :],
                                        in1=sw['xst'][:, t, 0:512], op=MULT)
                            elif nm == 'p0':
                                eng.scalar_tensor_tensor(
                                    out=msg[:, t, 0:24].rearrange(
                                        "p (h i) -> p h i", i=8),
                                    in0=scps[:, 0:24].rearrange(
                                        "p (h i) -> p h i", i=8),
                                    scalar=0.0,
                                    in1=sw['xst'][:, t, 0:8].unsqueeze(1)
                                    .broadcast_to([128, 3, 8]),
                                    op0=MAX, op1=MULT)
                            else:
                                # tail: na (3) [+ fyo (9) before na for c0]
                                if nm == 'c0':
                                    eng.scalar_tensor_tensor(
                                        out=msg[:, t, dmain:dmain + 9].rearrange(
                                            "p (f h) -> p f h", h=3),
                                        in0=scps[:, 0:9].rearrange(
                                            "p (f h) -> p f h", h=3),
                                        scalar=0.0,
                                        in1=fyo_sb[:, 3 * tg:3 * tg + 3]
                                        .unsqueeze(2).broadcast_to([128, 3, 3]),
                                        op0=MAX, op1=MULT)
                                    nlo, ntw = 9, 3
                                else:
                                    nlo, ntw = 0, TWL
                                eng.scalar_tensor_tensor(
                                    out=msg[:, t, dmain + nlo:dmain + nlo + ntw],
                                    in0=scps[:, nlo:nlo + ntw],
                                    scalar=0.0,
                                    in1=na_sb[:, tg:tg + 1].broadcast_to(
                                        [128, ntw]),
                                    op0=MAX, op1=MULT)
                    emit_scat(10000)

                def st_evac(w):
                    ssw = state[w]
                    ags = agsp.tile([128, DMAX], F16, tag="ags")
                    nc.scalar.activation(ags[:, 0:D], ssw['agps'][:, 0:D], COPY)
                    ssw['ags'] = ags

                def st_transpose(w):
                    ssw = state[w]
                    ags = ssw['ags']
                    agT = agtp.tile([128, 12 * 128], F16, tag="agT")
                    agTt = agtp.tile([32, 128], F16, tag="agTt")
                    for gi in range(0, nfull, 8):
                        gn = min(8, nfull - gi)
                        tp = ps_tp.tile([128, 1024], F16, tag="tp")
                        for u in range(gn):
                            ci = gi + u
                            nc.tensor.transpose(
                                tp[:, u * 128:(u + 1) * 128],
                                ags[:, ci * 128:(ci + 1) * 128],
                                ident[:, :])
                        eng = nc.vector if gi == 0 else nc.scalar
                        if eng is nc.vector:
                            nc.vector.tensor_copy(
                                agT[:, gi * 128:(gi + gn) * 128],
                                tp[:, 0:gn * 128])
                        else:
                            nc.scalar.activation(
                                agT[:, gi * 128:(gi + gn) * 128],
                                tp[:, 0:gn * 128], COPY)
                    tp2 = ps_tp.tile([128, 1024], F16, tag="tp")
                    nc.tensor.transpose(
                        tp2[0:TWL, 0:128],
                        ags[:, dmain:dmain + TWL],
                        ident[:, :])
                    nc.vector.tensor_copy(agTt[0:TWL, :], tp2[0:TWL, 0:128])
                    ssw['agT'] = agT
                    ssw['agTt'] = agTt

                def st_outmm(w, wa, k):
                    ssw = state[w]
                    wsz = cfg.WSIZES[w]
                    omf = ps_tp.tile([128, 512], F32, tag="tp")
                    for ci in range(nfull):
                        nc.tensor.matmul(
                            omf[:, 0:OCP],
                            ssw['agT'][:, ci * 128:(ci + 1) * 128],
                            wt[:, ci * OCP:(ci + 1) * OCP],
                            start=(ci == 0), stop=False)
                    nc.tensor.matmul(omf[:, 0:OCP], ssw['agTt'][0:TWL, :],
                                     wtail[0:TWL, 0:OCP],
                                     start=(nfull == 0), stop=False)
                    nc.tensor.matmul(omf[:, 0:OCP], ones[:, :],
                                     wbias[:, 0:OCP], start=False, stop=True)
                    odt = F32 if nm == 'c2' else F16
                    outs = outp.tile([128, 512], odt, tag="outs")
                    nc.scalar.activation(outs[:, 0:OCP], omf[:, 0:OCP], TANH)
                    if nm == 'c2':
                        nc.sync.dma_start(
                            out=out_fin[w * 128:w * 128 + wsz, :],
                            in_=outs[0:wsz, 0:cfg.OUT])
                    else:
                        outr = outp.tile([128, 512], F16, tag="outr")
                        nc.vector.tensor_tensor(out=outr[:, 0:OCP],
                                                in0=outs[:, 0:OCP],
                                                in1=zeros[:, 0:OCP], op=MAX)
                        ro = sum(cfg.WSIZES[wa:w])
                        nc.sync.dma_start(
                            out=oslices[k][ro:ro + wsz, :],
                            in_=outr[0:wsz, 0:HID])
                    del state[w]

                # window -> chunk lookup
                w_chunk = {}
                for k, (wa, wb) in enumerate(cfg.WCHUNKS):
                    for w in range(wa, wb):
                        w_chunk[w] = (k, wa, wb)

                for it in range(NW + 4):
                    if 0 <= it - 3 < NW:
                        st_transpose(it - 3)
                    if 0 <= it - 4 < NW:
                        w4 = it - 4
                        k, wa, wb = w_chunk[w4]
                        st_outmm(w4, wa, k)
                        # AllGather when the chunk's last window retires
                        if nm != 'c2' and w4 == wb - 1:
                            r = cfg.CROWS[k]
                            nc.gpsimd.collective_compute(
                                "AllGather", mybir.AluOpType.bypass,
                                replica_groups=[list(range(NCORES))],
                                ins=[oslices[k][:, :]],
                                outs=[Xout[cbase[k]:cbase[k] + NCORES * r, :]])
                    w1 = it - 1 if 0 <= it - 1 < NW else None
                    w2 = it - 2 if 0 <= it - 2 < NW else None
                    if w1 is not None or w2 is not None:
                        st_scaling_msg(w1, w2)
                    if w2 is not None:
                        st_evac(w2)
                    if it == 0:
                        st_gather(0)
                    if it + 1 < NW:
                        st_gather(it + 1)

            for lay in cfg.LAYERS:
                emit_layer(lay)
    nc.compile()
    return nc


def _run(inputs, trace=False):
    cfg = Cfg()
    struct, per_core, wts, xc0, asm = _preprocess(cfg, inputs)
    nc = _build(cfg, struct)
    in_maps = []
    for c in range(NCORES):
        im = dict(per_core[c])
        im['xc0_in'] = xc0
        for k, v in wts.items():
            im[k] = v
        in_maps.append(im)
    res = run_bass_kernel_spmd(nc, in_maps, list(range(NCORES)), trace=trace)
    out = np.zeros((cfg.N, cfg.OUT), np.float32)
    for c in range(NCORES):
        sl = res.results[c]['out_final']
        sel = asm['node_core'] == c
        out[sel] = sl[asm['node_row'][sel]]
    return out, res


def kernel(**inputs):
    return _run(inputs, trace=False)[0]


# revision 20
# speedup vs baseline: 2.0778x; 1.1359x over previous
"""Trainium2 Bass kernel for nn_CFDFVnewGCN (6-layer FVnewConv GNN).

Strategy: shard destination nodes (and their incoming edges) across 8 cores.
Nodes are permuted/degree-balanced into 49 windows of <=128 nodes per core.
fp16 datapath end-to-end (PSUM accumulation stays f32).  Per 128-edge tile:
scaling matmul (edge_attr stationary, bias folded as 8th K-row) into PSUM,
fused relu*x_j via scalar_tensor_tensor (PSUM -> fp16 msg, split between DVE
and GPSIMD), one-hot scatter matmul accumulating aggr[window, D] in PSUM.
Per window: PE transposes of aggr + output matmul (bias via const ones row),
tanh (ACT) + relu (DVE).  d-layout: 3 planes of 512 gathered x cols
(d = h*512 + i), then a small tail (na, and fyo for c0) so the scatter PSUM
is exactly 4 banks.  Software pipeline: at iteration `it` we emit
gather(it) / scaling+msg(it-1) interleaved with scatter(it-2) /
transposes(it-3) / out-matmul(it-4), keeping the PE stream dependency-free
and continuously busy (full 2.4 GHz p-state).  x replicated via 4 chunked
AllGathers per layer (overlapping compute) into a Shared DRAM buffer.
Gather uses dma_gather with two offset views of the x buffer (rows 0:32768
and 17232:50000) so indices fit int16.
"""
import sys
import numpy as np

for _p in ('/opt/trn_rl_repo', '/root/.axon_site/_ro/trn_rl_repo'):
    if _p not in sys.path:
        sys.path.insert(0, _p)

import concourse.bacc as bacc
import concourse.bass as bass
import concourse.mybir as mybir
import concourse.tile as tile
from concourse.bass_utils import run_bass_kernel_spmd

F32 = mybir.dt.float32
F16 = mybir.dt.float16
I16 = mybir.dt.int16
I32 = mybir.dt.int32
COPY = mybir.ActivationFunctionType.Copy
RELU = mybir.ActivationFunctionType.Relu
TANH = mybir.ActivationFunctionType.Tanh
MULT = mybir.AluOpType.mult
MAX = mybir.AluOpType.max
ISEQ = mybir.AluOpType.is_equal

NCORES = 8
G = 512          # gathered x columns (one plane width)
DMAIN = 3 * G    # 1536


class Cfg:
    def __init__(self, n_nodes=50000, n_edges=200000, hid=512, hs=3, ea=6,
                 out=3, n_ag_chunks=4):
        self.N = n_nodes
        self.E = n_edges
        self.HID = hid
        self.HS = hs
        self.EA = ea
        self.OUT = out
        self.NPC = self.N // NCORES              # nodes per core
        self.NWIN = (self.NPC + 127) // 128      # windows per core
        self.WSIZES = [128] * (self.NWIN - 1) + [self.NPC - 128 * (self.NWIN - 1)]
        # A/B view split of the x buffer rows (int16 gather index range)
        self.VIEW = min(32768, self.N)
        self.ABOFS = max(0, self.N - self.VIEW)
        # AllGather chunking: split windows into n_ag_chunks groups
        if n_ag_chunks == 4 and self.NWIN == 49:
            bounds = [0, 20, 35, 45, 49]
        else:
            k = min(n_ag_chunks, self.NWIN)
            bounds = [round(i * self.NWIN / k) for i in range(k + 1)]
        self.WCHUNKS = [(bounds[i], bounds[i + 1]) for i in range(len(bounds) - 1)]
        self.CROWS = [sum(self.WSIZES[a:b]) for a, b in self.WCHUNKS]
        # layer table
        #  p0: ic=7  ([x5,sdf,na] + pad), gathered from xc0 (16 fp16 cols)
        #  others: 3 planes of 512 x-cols + tail (na=3; c0 adds fyo=9)
        self.LAYERS = []
        for name in ['p0', 'p1', 'p2', 'c0', 'c1', 'c2']:
            if name == 'p0':
                lay = dict(name=name, oc=hid, dmain=0, tw=24, gcols=128,
                           relu=True)
            else:
                oc = out if name == 'c2' else hid
                tw = 12 if name == 'c0' else 4
                lay = dict(name=name, oc=oc, dmain=DMAIN, tw=tw, gcols=G,
                           relu=(name != 'c2'))
            lay['D'] = lay['dmain'] + lay['tw']
            lay['OCP'] = lay['oc'] + (-lay['oc']) % 4
            self.LAYERS.append(lay)


def _col2orig(cfg, lay):
    """Map new d-layout column -> original scaling index j=i*HS+h, -1=pad."""
    HS = cfg.HS
    m = np.full(lay['D'], -1, np.int64)
    nm = lay['name']
    if nm == 'p0':
        # d = h*8 + i, i<7 ([x5, sdf, na])
        for h in range(HS):
            for i in range(7):
                m[h * 8 + i] = i * HS + h
    else:
        # ic layout in reference xc: c0: [fyo3, x512, na1]; else [x512, na1]
        xoff = 3 if nm == 'c0' else 0
        for h in range(HS):
            for i in range(G):
                m[h * G + i] = (xoff + i) * HS + h
        t0 = DMAIN
        if nm == 'c0':
            for f in range(3):
                for h in range(HS):
                    m[t0 + 3 * f + h] = f * HS + h        # fyo
            for h in range(HS):
                m[t0 + 9 + h] = (3 + G) * HS + h          # na
        else:
            for h in range(HS):
                m[t0 + h] = G * HS + h                    # na
    return m


def _balance(items_deg, caps):
    """Greedy: assign items (sorted by degree desc) to bins with capacity,
    minimizing max degree sum. Returns bin index per item."""
    order = np.argsort(-items_deg, kind='stable')
    nbins = len(caps)
    load = np.zeros(nbins)
    cnt = np.zeros(nbins, np.int64)
    out = np.zeros(len(items_deg), np.int64)
    import heapq
    heap = [(0.0, b) for b in range(nbins)]
    heapq.heapify(heap)
    for it in order:
        while True:
            l, b = heapq.heappop(heap)
            if cnt[b] < caps[b]:
                break
        out[it] = b
        cnt[b] += 1
        load[b] += items_deg[it]
        if cnt[b] < caps[b]:
            heapq.heappush(heap, (load[b], b))
    return out


def _preprocess(cfg, inputs):
    N, E, HS = cfg.N, cfg.E, cfg.HS
    ei = np.asarray(inputs['edge_index'])
    src = ei[0].astype(np.int64)
    dst = ei[1].astype(np.int64)
    deg = np.bincount(dst, minlength=N).astype(np.float64)

    node_core = _balance(deg, [cfg.NPC] * NCORES)
    node_win = np.zeros(N, np.int64)
    node_slot = np.zeros(N, np.int64)
    for c in range(NCORES):
        nodes = np.where(node_core == c)[0]
        w = _balance(deg[nodes], cfg.WSIZES)
        node_win[nodes] = w
        for wi in range(cfg.NWIN):
            sel = nodes[w == wi]
            node_slot[sel] = np.arange(len(sel))

    # within-core row and global x row (AG chunk-major, rank-interleaved)
    node_row = node_win * 128 + node_slot
    cbase = np.concatenate([[0], np.cumsum([r * NCORES for r in cfg.CROWS])])
    wchunk = np.zeros(cfg.NWIN, np.int64)
    wofs = np.zeros(cfg.NWIN, np.int64)
    for k, (a, b) in enumerate(cfg.WCHUNKS):
        for w in range(a, b):
            wchunk[w] = k
            wofs[w] = sum(cfg.WSIZES[a:w])
    k_of = wchunk[node_win]
    xrow = (cbase[k_of] + node_core * np.array(cfg.CROWS)[k_of]
            + wofs[node_win] + node_slot)
    xrow_src = xrow[src]

    # edge buckets per (core, window)
    ec = node_core[dst]
    ew = node_win[dst]

    # per-window global tile structure (max over cores)
    tw = np.zeros(cfg.NWIN, np.int64)
    cntT = np.zeros((NCORES, cfg.NWIN), np.int64)
    np.add.at(cntT, (ec, ew), 1)
    for w in range(cfg.NWIN):
        tw[w] = max(int(np.ceil(cntT[:, w].max() / 128)), 1)
    tbase = np.concatenate([[0], np.cumsum(tw)])
    T = int(tbase[-1])

    ea_np = np.asarray(inputs['edge_attr'], np.float32)
    na_np = np.asarray(inputs['node_attr'], np.float32).reshape(-1)
    fyo_np = np.asarray(inputs['fine_y_orig'], np.float32)

    per_core = []
    for c in range(NCORES):
        ea_s = np.zeros((7, T * 128), np.float16)
        idx_s = np.zeros((128, T), np.int32)
        dst_s = np.full((128, T), 999.0, np.float16)
        na_s = np.zeros((128, T), np.float16)
        fyo_s = np.zeros((128, 3 * T), np.float16)
        for w in range(cfg.NWIN):
            edges = np.where((ec == c) & (ew == w))[0]
            t0 = int(tbase[w])
            assert len(edges) <= tw[w] * 128
            jj = np.arange(len(edges))
            e_tt = t0 + jj // 128
            e_pp = jj % 128
            idx_s[e_pp, e_tt] = xrow_src[edges].astype(np.int32)
            ea_s[0:6, e_tt * 128 + e_pp] = ea_np[edges].T
            ea_s[6, e_tt * 128 + e_pp] = 1.0
            dst_s[e_pp, e_tt] = node_slot[dst[edges]]
            na_s[e_pp, e_tt] = na_np[src[edges]]
            fyo_s[e_pp.repeat(3), (e_tt * 3).repeat(3)
                  + np.tile([0, 1, 2], len(edges))] = fyo_np[src[edges]].ravel()
        per_core.append(dict(ea_s=ea_s, idx_s=idx_s,
                             dst_s=dst_s, na_s=na_s, fyo_s=fyo_s))

    # xc0 buffer: [N, 16] fp16 in x-row order: cols [x(5), sdf, na, 0...]
    x_np = np.asarray(inputs['x'], np.float32)
    sdf_np = np.asarray(inputs['sdf'], np.float32)
    xc0 = np.zeros((N, 128), np.float16)
    xc0[xrow, 0:x_np.shape[1]] = x_np
    xc0[xrow, x_np.shape[1]] = sdf_np[:, 0]
    xc0[xrow, x_np.shape[1] + 1] = na_np

    # weights per layer (fp16, reordered to kernel d-layout)
    wts = {}
    for lay in cfg.LAYERS:
        nm = lay['name']
        win = np.asarray(inputs[f'win_{nm}'], np.float32)
        bin_ = np.asarray(inputs[f'bin_{nm}'], np.float32)
        wout = np.asarray(inputs[f'wout_{nm}'], np.float32)
        bout = np.asarray(inputs[f'bout_{nm}'], np.float32)
        m = _col2orig(cfg, lay)
        D, OCP = lay['D'], lay['OCP']
        winT = np.zeros((7, D), np.float16)
        sel = m >= 0
        winT[0:cfg.EA, sel] = win[m[sel]].T
        winT[6, sel] = bin_[m[sel]]
        woutT = np.zeros((D + 1, OCP), np.float16)
        woutT[np.where(sel)[0], 0:lay['oc']] = wout[:, m[sel]].T
        woutT[D, 0:lay['oc']] = bout
        wts[f'winT_{nm}'] = winT
        wts[f'woutT_{nm}'] = woutT

    struct = dict(tw=tw, tbase=tbase, T=T, TWMAX=int(tw.max()))
    asm = dict(node_core=node_core, node_row=node_row)
    return struct, per_core, wts, xc0, asm


def _build(cfg, struct):
    twin, tbase, T = struct['tw'], struct['tbase'], struct['T']
    TWMAX = struct['TWMAX']
    HID = cfg.HID
    NW = cfg.NWIN
    DMAX = max(l['D'] for l in cfg.LAYERS)

    nc = bacc.Bacc("TRN2", target_bir_lowering=False, debug=False,
                   enable_asserts=True, num_devices=NCORES,
                   num_swdge_queues=4)
    ea_in = nc.dram_tensor("ea_s", [7, T * 128], F16, kind="ExternalInput").ap()
    idx_in = nc.dram_tensor("idx_s", [128, T], I32, kind="ExternalInput").ap()
    dst_in = nc.dram_tensor("dst_s", [128, T], F16, kind="ExternalInput").ap()
    na_in = nc.dram_tensor("na_s", [128, T], F16, kind="ExternalInput").ap()
    fyo_in = nc.dram_tensor("fyo_s", [128, 3 * T], F16, kind="ExternalInput").ap()
    xc0_in = nc.dram_tensor("xc0_in", [cfg.N, 128], F16, kind="ExternalInput").ap()
    win_ins = {}
    wout_ins = {}
    for lay in cfg.LAYERS:
        nm = lay['name']
        win_ins[nm] = nc.dram_tensor(f"winT_{nm}", [7, lay['D']], F16,
                                     kind="ExternalInput").ap()
        wout_ins[nm] = nc.dram_tensor(f"woutT_{nm}", [lay['D'] + 1, lay['OCP']],
                                      F16, kind="ExternalInput").ap()
    out_fin = nc.dram_tensor("out_final", [cfg.NPC, cfg.OUT], F32,
                             kind="ExternalOutput").ap()

    with tile.TileContext(nc) as tc:
        with (
            tc.tile_pool(name="cst", bufs=1) as cst,
            tc.tile_pool(name="sbw", bufs=2) as sbw,
            tc.tile_pool(name="gst", bufs=4) as gst,
            tc.tile_pool(name="eap", bufs=4) as eap,
            tc.tile_pool(name="msgp", bufs=3) as msgp,
            tc.tile_pool(name="scp", bufs=3) as scp,
            tc.tile_pool(name="Sp", bufs=4) as Sp,
            tc.tile_pool(name="agsp", bufs=3) as agsp,
            tc.tile_pool(name="agtp", bufs=3) as agtp,
            tc.tile_pool(name="outp", bufs=2) as outp,
            tc.tile_pool(name="ps_sc", bufs=3, space="PSUM") as ps_sc,
            tc.tile_pool(name="ps_ag", bufs=1, space="PSUM") as ps_ag,
            tc.tile_pool(name="ps_tp", bufs=1, space="PSUM") as ps_tp,
            tc.tile_pool(name="dram", bufs=1, space="DRAM") as dram,
        ):
            # ---- constants
            iota_i = cst.tile([128, 128], I32)
            nc.gpsimd.iota(iota_i[:, :], pattern=[[1, 128]], base=0,
                           channel_multiplier=0)
            iota_f = cst.tile([128, 128], F16)
            nc.vector.tensor_copy(iota_f[:, :], iota_i[:, :])
            iota_p = cst.tile([128, 1], I32)
            nc.gpsimd.iota(iota_p[:, :], pattern=[[1, 1]], base=0,
                           channel_multiplier=1)
            iota_pf = cst.tile([128, 1], F32)
            nc.vector.tensor_copy(iota_pf[:, :], iota_p[:, :])
            identf = cst.tile([128, 128], F32)
            nc.vector.tensor_scalar(out=identf[:, :], in0=iota_f[:, :],
                                    scalar1=iota_pf[:, :], scalar2=None,
                                    op0=ISEQ)
            ident = cst.tile([128, 128], F16)
            nc.vector.tensor_copy(ident[:, :], identf[:, :])
            ones_i = cst.tile([1, 128], I32)
            nc.gpsimd.iota(ones_i[:, :], pattern=[[0, 128]], base=1,
                           channel_multiplier=0)
            ones = cst.tile([1, 128], F16)
            nc.vector.tensor_copy(ones[:, :], ones_i[:, :])
            zeros = cst.tile([128, 512], F16)
            nc.vector.memset(zeros[:, :], 0)

            # ---- static per-slot data (resident)
            dst_sb = cst.tile([128, T], F16)
            nc.sync.dma_start(out=dst_sb[:, :], in_=dst_in[:, :])
            na_sb = cst.tile([128, T], F16)
            nc.sync.dma_start(out=na_sb[:, :], in_=na_in[:, :])
            fyo_sb = cst.tile([128, 3 * T], F16)
            nc.sync.dma_start(out=fyo_sb[:, :], in_=fyo_in[:, :])
            idx_sb = cst.tile([128, T], I32)
            nc.sync.dma_start(out=idx_sb[:, :], in_=idx_in[:, :])

            # ---- DRAM buffers
            xc0b = dram.tile([cfg.N, 128], F16)
            nc.sync.dma_start(out=xc0b[:, :], in_=xc0_in[:, :])
            xstate = {}
            cbase = np.concatenate(
                [[0], np.cumsum([r * NCORES for r in cfg.CROWS])]).astype(int)

            def emit_layer(lay):
                nm, D, OCP, TWL = lay['name'], lay['D'], lay['OCP'], lay['tw']
                dmain = lay['dmain']
                gcols = lay['gcols']
                nfull = dmain // 128                     # 12 or 0
                # out-mm K chunks: nfull x 128 + tail TWL
                if nm == 'p0':
                    gsrc = xc0b
                else:
                    gsrc = xstate['cur']

                # layer weights (fp16 direct)
                winT = sbw.tile([7, DMAX], F16, tag="winT")
                nc.sync.dma_start(out=winT[:, 0:D], in_=win_ins[nm][:, :])
                wt = sbw.tile([128, 12 * 512], F16, tag="wt")
                for ci in range(nfull):
                    nc.sync.dma_start(
                        out=wt[:, ci * OCP:ci * OCP + OCP],
                        in_=wout_ins[nm][ci * 128:(ci + 1) * 128, :])
                wtail = sbw.tile([32, 512], F16, tag="wtail")
                nc.sync.dma_start(
                    out=wtail[0:TWL, 0:OCP],
                    in_=wout_ins[nm][nfull * 128:nfull * 128 + TWL, :])
                wbias = sbw.tile([1, 512], F16, tag="wbias")
                nc.sync.dma_start(out=wbias[:, 0:OCP],
                                  in_=wout_ins[nm][D:D + 1, :])

                if nm != 'c2':
                    Xout = dram.tile([cfg.N, HID], F16,
                                     tag="Xbuf", name=f"X_{nm}", bufs=2)
                    oslices = []
                    for k, r in enumerate(cfg.CROWS):
                        t_ = dram.tile([r, HID], F16, tag=f"osl_{k}",
                                       name=f"osl_{nm}_{k}", bufs=1)
                        oslices.append(t_)
                    xstate['cur'] = Xout

                # pipeline state per window
                state = {}

                def st_gather(w):
                    nt = int(twin[w])
                    t0 = int(tbase[w])
                    xst = gst.tile([128, TWMAX, gcols], F16, tag=f"xst{gcols}")
                    for t in range(nt):
                        nc.gpsimd.indirect_dma_start(
                            out=xst[:, t, 0:gcols],
                            out_offset=None,
                            in_=gsrc[:, :],
                            in_offset=bass.IndirectOffsetOnAxis(
                                ap=idx_sb[:, t0 + t:t0 + t + 1], axis=0))
                    eaf = eap.tile([7, TWMAX * 128], F16, tag="eaf")
                    nc.sync.dma_start(out=eaf[:, 0:nt * 128],
                                      in_=ea_in[:, t0 * 128:(t0 + nt) * 128])
                    S = Sp.tile([128, TWMAX * 128], F16, tag="S")
                    nc.vector.tensor_tensor(
                        out=S[:, 0:nt * 128].rearrange("p (t n) -> p t n", n=128),
                        in0=iota_f[:, :].unsqueeze(1).broadcast_to([128, nt, 128]),
                        in1=dst_sb[:, t0:t0 + nt].unsqueeze(2).broadcast_to(
                            [128, nt, 128]),
                        op=ISEQ)
                    state[w] = dict(xst=xst, eaf=eaf, S=S)

                def st_scaling_msg(w, scat_w):
                    """Interleave scaling MMs + fused relu-mult for window w
                    with scatter MMs for window scat_w (PE never waits)."""
                    nt = int(twin[w]) if w is not None else 0
                    t0 = int(tbase[w]) if w is not None else 0
                    if w is not None:
                        sw = state[w]
                        msg = msgp.tile([128, TWMAX, DMAX], F16, tag="msg")
                        sw['msg'] = msg
                    # scatter stream for scat_w
                    scat_ops = []
                    if scat_w is not None:
                        ssw = state[scat_w]
                        snt = int(twin[scat_w])
                        agps = ps_ag.tile([128, 2048], F32, tag="agps")
                        ssw['agps'] = agps
                        smsg = ssw['msg']
                        sS = ssw['S']
                        npieces = (3 if dmain else 0) + 1
                        for t in range(snt):
                            for pi in range(npieces):
                                if dmain and pi < 3:
                                    lo, hi = pi * 512, (pi + 1) * 512
                                else:
                                    lo, hi = dmain, dmain + TWL
                                scat_ops.append((t, lo, hi))
                        scat_i = [0]

                        def emit_scat(n=1):
                            for _ in range(n):
                                if scat_i[0] >= len(scat_ops):
                                    return
                                t, lo, hi = scat_ops[scat_i[0]]
                                scat_i[0] += 1
                                nc.tensor.matmul(
                                    agps[:, lo:hi],
                                    sS[:, t * 128:(t + 1) * 128],
                                    smsg[:, t, lo:hi],
                                    start=(t == 0), stop=(t == snt - 1))
                    else:
                        def emit_scat(n=1):
                            return

                    # scaling + fused relu*x for w, interleaved
                    eng_i = [0]
                    for t in range(nt):
                        tg = t0 + t
                        pieces = ([(p * 512, (p + 1) * 512) for p in range(3)]
                                  if dmain else [])
                        pieces.append((dmain, dmain + TWL))
                        for (lo, hi) in pieces:
                            scps = ps_sc.tile([128, 512], F32, tag="scps")
                            nc.tensor.matmul(
                                scps[:, 0:hi - lo],
                                sw['eaf'][:, t * 128:(t + 1) * 128],
                                winT[:, lo:hi], start=True, stop=True)
                            emit_scat(1)
                            # msg = relu(scps) * x_j  (GPSIMD can't touch
                            # PSUM: path A = DVE fused STT; B/C = ACT relu
                            # then DVE-2x / GPSIMD multiply)
                            eng = nc.vector
                            if dmain and hi <= dmain:
                                path = 'ACB'[eng_i[0] % 3]
                                eng_i[0] += 1
                                if path == 'A':
                                    nc.vector.scalar_tensor_tensor(
                                        out=msg[:, t, lo:hi],
                                        in0=scps[:, 0:512],
                                        scalar=0.0, in1=sw['xst'][:, t, 0:512],
                                        op0=MAX, op1=MULT)
                                else:
                                    sc = scp.tile([128, 512], F16, tag="sc")
                                    nc.scalar.activation(sc[:, :],
                                                         scps[:, 0:512], RELU)
                                    meng = nc.vector if path == 'B' else nc.gpsimd
                                    meng.tensor_tensor(
                                        out=msg[:, t, lo:hi], in0=sc[:, :],
                                        in1=sw['xst'][:, t, 0:512], op=MULT)
                            elif nm == 'p0':
                                eng.scalar_tensor_tensor(
                                    out=msg[:, t, 0:24].rearrange(
                                        "p (h i) -> p h i", i=8),
                                    in0=scps[:, 0:24].rearrange(
                                        "p (h i) -> p h i", i=8),
                                    scalar=0.0,
                                    in1=sw['xst'][:, t, 0:8].unsqueeze(1)
                                    .broadcast_to([128, 3, 8]),
                                    op0=MAX, op1=MULT)
                            else:
                                # tail: na (3) [+ fyo (9) before na for c0]
                                if nm == 'c0':
                                    eng.scalar_tensor_tensor(
                                        out=msg[:, t, dmain:dmain + 9].rearrange(
                                            "p (f h) -> p f h", h=3),
                                        in0=scps[:, 0:9].rearrange(
                                            "p (f h) -> p f h", h=3),
                                        scalar=0.0,
                                        in1=fyo_sb[:, 3 * tg:3 * tg + 3]
                                        .unsqueeze(2).broadcast_to([128, 3, 3]),
                                        op0=MAX, op1=MULT)
                                    nlo, ntw = 9, 3
                                else:
                                    nlo, ntw = 0, TWL
                                eng.scalar_tensor_tensor(
                                    out=msg[:, t, dmain + nlo:dmain + nlo + ntw],
                                    in0=scps[:, nlo:nlo + ntw],
                                    scalar=0.0,
                                    in1=na_sb[:, tg:tg + 1].broadcast_to(
                                        [128, ntw]),
                                    op0=MAX, op1=MULT)
                    emit_scat(10000)

                def st_evac(w):
                    ssw = state[w]
                    ags = agsp.tile([128, DMAX], F16, tag="ags")
                    nc.scalar.activation(ags[:, 0:D], ssw['agps'][:, 0:D], COPY)
                    ssw['ags'] = ags

                def st_transpose(w):
                    ssw = state[w]
                    ags = ssw['ags']
                    agT = agtp.tile([128, 12 * 128], F16, tag="agT")
                    agTt = agtp.tile([32, 128], F16, tag="agTt")
                    for gi in range(0, nfull, 8):
                        gn = min(8, nfull - gi)
                        tp = ps_tp.tile([128, 1024], F16, tag="tp")
                        for u in range(gn):
                            ci = gi + u
                            nc.tensor.transpose(
                                tp[:, u * 128:(u + 1) * 128],
                                ags[:, ci * 128:(ci + 1) * 128],
                                ident[:, :])
                        eng = nc.vector if gi == 0 else nc.scalar
                        if eng is nc.vector:
                            nc.vector.tensor_copy(
                                agT[:, gi * 128:(gi + gn) * 128],
                                tp[:, 0:gn * 128])
                        else:
                            nc.scalar.activation(
                                agT[:, gi * 128:(gi + gn) * 128],
                                tp[:, 0:gn * 128], COPY)
                    tp2 = ps_tp.tile([128, 1024], F16, tag="tp")
                    nc.tensor.transpose(
                        tp2[0:TWL, 0:128],
                        ags[:, dmain:dmain + TWL],
                        ident[:, :])
                    nc.vector.tensor_copy(agTt[0:TWL, :], tp2[0:TWL, 0:128])
                    ssw['agT'] = agT
                    ssw['agTt'] = agTt

                def st_outmm(w, wa, k):
                    ssw = state[w]
                    wsz = cfg.WSIZES[w]
                    omf = ps_tp.tile([128, 512], F32, tag="tp")
                    for ci in range(nfull):
                        nc.tensor.matmul(
                            omf[:, 0:OCP],
                            ssw['agT'][:, ci * 128:(ci + 1) * 128],
                            wt[:, ci * OCP:(ci + 1) * OCP],
                            start=(ci == 0), stop=False)
                    nc.tensor.matmul(omf[:, 0:OCP], ssw['agTt'][0:TWL, :],
                                     wtail[0:TWL, 0:OCP],
                                     start=(nfull == 0), stop=False)
                    nc.tensor.matmul(omf[:, 0:OCP], ones[:, :],
                                     wbias[:, 0:OCP], start=False, stop=True)
                    odt = F32 if nm == 'c2' else F16
                    outs = outp.tile([128, 512], odt, tag="outs")
                    nc.scalar.activation(outs[:, 0:OCP], omf[:, 0:OCP], TANH)
                    if nm == 'c2':
                        nc.sync.dma_start(
                            out=out_fin[w * 128:w * 128 + wsz, :],
                            in_=outs[0:wsz, 0:cfg.OUT])
                    else:
                        outr = outp.tile([128, 512], F16, tag="outr")
                        nc.vector.tensor_tensor(out=outr[:, 0:OCP],
                                                in0=outs[:, 0:OCP],
                                                in1=zeros[:, 0:OCP], op=MAX)
                        ro = sum(cfg.WSIZES[wa:w])
                        nc.sync.dma_start(
                            out=oslices[k][ro:ro + wsz, :],
                            in_=outr[0:wsz, 0:HID])
                    del state[w]

                # window -> chunk lookup
                w_chunk = {}
                for k, (wa, wb) in enumerate(cfg.WCHUNKS):
                    for w in range(wa, wb):
                        w_chunk[w] = (k, wa, wb)

                for it in range(NW + 4):
                    w1 = it - 1 if 0 <= it - 1 < NW else None
                    w2 = it - 2 if 0 <= it - 2 < NW else None
                    if w1 is not None or w2 is not None:
                        st_scaling_msg(w1, w2)
                    if w2 is not None:
                        st_evac(w2)
                    if 0 <= it - 3 < NW:
                        st_transpose(it - 3)
                    if 0 <= it - 4 < NW:
                        w4 = it - 4
                        k, wa, wb = w_chunk[w4]
                        st_outmm(w4, wa, k)
                        # AllGather when the chunk's last window retires
                        if nm != 'c2' and w4 == wb - 1:
                            r = cfg.CROWS[k]
                            nc.gpsimd.collective_compute(
                                "AllGather", mybir.AluOpType.bypass,
                                replica_groups=[list(range(NCORES))],
                                ins=[oslices[k][:, :]],
                                outs=[Xout[cbase[k]:cbase[k] + NCORES * r, :]])
                    if it == 0:
                        st_gather(0)
                    if it + 1 < NW:
                        st_gather(it + 1)

            for lay in cfg.LAYERS:
                emit_layer(lay)
    nc.compile()
    return nc


def _run(inputs, trace=False):
    cfg = Cfg()
    struct, per_core, wts, xc0, asm = _preprocess(cfg, inputs)
    nc = _build(cfg, struct)
    in_maps = []
    for c in range(NCORES):
        im = dict(per_core[c])
        im['xc0_in'] = xc0
        for k, v in wts.items():
            im[k] = v
        in_maps.append(im)
    res = run_bass_kernel_spmd(nc, in_maps, list(range(NCORES)), trace=trace)
    out = np.zeros((cfg.N, cfg.OUT), np.float32)
    for c in range(NCORES):
        sl = res.results[c]['out_final']
        sel = asm['node_core'] == c
        out[sel] = sl[asm['node_row'][sel]]
    return out, res


def kernel(**inputs):
    return _run(inputs, trace=False)[0]
